# revision 16
# baseline (speedup 1.0000x reference)
"""2-layer GraphSAGE (mean aggregation) on 8 Trainium2 NeuronCores — v2.

Strategy (dst-sharded graph parallel), changes vs v1:
- bf16 data path: x table, staged gathers, indicators, weight matmuls (PE
  1cyc/row vs fp32 4), PSUM accumulate fp32.
- Transposed accumulation: acc[F, dst] = stage[pos, F].T @ ind[pos, dst];
  1/deg folded into the indicator value (tensor_scalar is_equal * rv), so
  PSUM holds mean^T directly -> no per-block scale/transpose chain.
- Superblock gather calls: ~4096 indices/call (vs 512) -> ~8x fewer SWDGE
  fixed overheads on Pool. Trailing pad indices are -1 (trimmed by ucode,
  no descriptors) instead of gathering row 0.
- Both layers share one idx/seg/rv tensor set (same edge structure, both
  tables are 256B-row bf16 [NG, 128]).
- Layer outputs are produced transposed [64, NL]; host untransposes.
"""
import sys
sys.path.insert(0, "/opt/trn_rl_repo")
import numpy as np

import concourse.bass as bass
import concourse.bacc as bacc
import concourse.mybir as mybir
import concourse.tile as tile
from concourse.bass_utils import run_bass_kernel_spmd

N_NODES = 100000
N_EDGES = 1600000
F_IN = 128
F_OUT = 64
P = 8
NREAL = 12500
NL = 12544            # 98 * 128
BLK = 128
NB = NL // BLK        # 98
SBK = 7               # blocks per superblock
NSUP = NB // SBK      # 14
SUPN = SBK * BLK      # 896 dsts per super
CHUNK = 32768
NCHUNK = (P * NL + CHUNK - 1) // CHUNK   # 4
NG = P * NL           # 100352
GCOLS = 8             # max 128-idx cols per gather call (1024 idxs).
                      # Empirical HW limits: 2048-idx calls deadlock the
                      # SWDGE ring (129 descs/engine > 128 in-flight cap);
                      # 1536/1920 also fail (Q7 idx scratch); 1024 is stable.
SENT = 999.0
AGS = 4                              # supers per AllGather group
GRP_ROWS = AGS * SUPN                # 3584
NGRP = (NSUP + AGS - 1) // AGS       # 4
GSZ = [min(NSUP, (g + 1) * AGS) * SUPN - g * GRP_ROWS for g in range(NGRP)]
GBASE = [0] * NGRP                   # global row base of each group
for _g in range(1, NGRP):
    GBASE[_g] = GBASE[_g - 1] + P * GSZ[_g - 1]


def _gmap(core, slot):
    # group-major global table row for (core, local slot)
    g = np.minimum(slot // GRP_ROWS, NGRP - 1)
    gsz = np.asarray(GSZ)[g]
    gbase = np.asarray(GBASE)[g]
    return gbase + core * gsz + (slot - g * GRP_ROWS)

BF16 = mybir.dt.bfloat16
NPBF16 = mybir.dt.np(BF16)


def _wrap16(flat_idx):
    w = flat_idx.reshape(-1, 16).T.copy()
    return np.tile(w, (8, 1))


def _preprocess(edge_index):
    src = np.asarray(edge_index[0], dtype=np.int64)
    dst = np.asarray(edge_index[1], dtype=np.int64)
    dcore = dst // NREAL
    dslot = dst - dcore * NREAL
    score = src // NREAL
    g_src = _gmap(score, src - score * NREAL)

    cores = []           # per core dict: ds, g (sorted), seg boundaries
    for k in range(P):
        sel = dcore == k
        ds = dslot[sel]
        g = g_src[sel]
        ch = g // CHUNK
        sup = ds // SUPN
        order = np.lexsort((g, ds, ch, sup))
        ds, g, ch, sup = ds[order], g[order], ch[order], sup[order]
        code = sup * NCHUNK + ch
        bounds = np.searchsorted(code, np.arange(NSUP * NCHUNK + 1))
        cnt = np.bincount(ds, minlength=NL).astype(np.float64)
        rv = (1.0 / np.maximum(cnt, 1.0)).astype(np.float32)
        cores.append(dict(ds=ds, g=g, bounds=bounds, rv=rv))

    # uniform cols per (sup, chunk)
    ncols = np.zeros((NSUP, NCHUNK), dtype=np.int64)
    for k in range(P):
        b = cores[k]["bounds"]
        n = (b[1:] - b[:-1]).reshape(NSUP, NCHUNK)
        ncols = np.maximum(ncols, (n + 127) // 128)
    seg_col0 = np.zeros((NSUP, NCHUNK), dtype=np.int64)   # global col base
    sup_col0 = np.zeros(NSUP, dtype=np.int64)             # col base within super
    tot = 0
    for s in range(NSUP):
        loc = 0
        for c in range(NCHUNK):
            seg_col0[s, c] = loc          # local to super
            loc += int(ncols[s, c])
        sup_col0[s] = tot
        tot += loc
    sup_cols = [int(ncols[s].sum()) for s in range(NSUP)]
    MAXSUPC = max(sup_cols)

    # calls: (s, c, loc_col0, cols, idx_off) — uniform
    calls = []
    idx_off = 0
    for s in range(NSUP):
        for c in range(NCHUNK):
            nc_ = int(ncols[s, c])
            done = 0
            while done < nc_:
                piece = min(GCOLS, nc_ - done)
                calls.append((s, c, int(seg_col0[s, c]) + done, piece, idx_off))
                idx_off += piece * 8
                done += piece
    IDX_COLS = idx_off

    # block windows (uniform): for each block b, chunk c -> [wlo, whi) local cols
    wins = np.zeros((NB, NCHUNK, 2), dtype=np.int64)
    wins[:, :, 0] = 1 << 60
    for k in range(P):
        ds, bounds = cores[k]["ds"], cores[k]["bounds"]
        for s in range(NSUP):
            for c in range(NCHUNK):
                s0, s1 = bounds[s * NCHUNK + c], bounds[s * NCHUNK + c + 1]
                blkseg = ds[s0:s1] // BLK
                for b in range(s * SBK, (s + 1) * SBK):
                    lo = int(np.searchsorted(blkseg, b))
                    hi = int(np.searchsorted(blkseg, b + 1))
                    if hi > lo:
                        wins[b, c, 0] = min(wins[b, c, 0], lo // 128)
                        wins[b, c, 1] = max(wins[b, c, 1], (hi + 127) // 128)

    # pairs: per block, list of (loc_col, pair_idx)
    pairs = [[] for _ in range(NB)]
    npair = 0
    for b in range(NB):
        s = b // SBK
        for c in range(NCHUNK):
            wlo, whi = wins[b, c]
            if whi <= wlo:
                continue
            for t in range(int(wlo), int(whi)):
                pairs[b].append((int(seg_col0[s, c]) + t, npair))
                npair += 1
    NPAIR = npair

    # per-core tensors
    idx_cores, seg_cores, rv_cores = [], [], []
    for k in range(P):
        ds, g, bounds, rv = (cores[k][x] for x in ("ds", "g", "bounds", "rv"))
        seg_np = np.full((NPAIR, 128), SENT, dtype=np.float32)
        rv_np = np.zeros((NPAIR, 128), dtype=np.float32)
        # pad positions gather row 0 (cheap, finite); sentinel seg zeroes
        # their contribution. Negative (skipped) indices desync the SWDGE
        # ring bookkeeping (decode reserves untrimmed, gen trims) -> hang.
        idx_flat = np.zeros(tot * 128, dtype=np.int64)
        for s in range(NSUP):
            for c in range(NCHUNK):
                s0, s1 = bounds[s * NCHUNK + c], bounds[s * NCHUNK + c + 1]
                n = s1 - s0
                base = (sup_col0[s] + seg_col0[s, c]) * 128
                idx_flat[base:base + n] = g[s0:s1] - c * CHUNK
        idx_cores.append(idx_flat)
        seg_cores.append(seg_np)
        rv_cores.append(rv_np)

    # fill seg/rv per pair (redo with pair indices known)
    pair_list = []   # (b, c, t_local)
    for b in range(NB):
        s = b // SBK
        for c in range(NCHUNK):
            wlo, whi = wins[b, c]
            for t in range(int(wlo), int(whi)):
                pair_list.append((b, c, t))
    assert len(pair_list) == NPAIR
    for k in range(P):
        ds, bounds, rv = (cores[k][x] for x in ("ds", "bounds", "rv"))
        seg_np = seg_cores[k]
        rv_np = rv_cores[k]
        for p, (b, c, t) in enumerate(pair_list):
            s = b // SBK
            s0, s1 = bounds[s * NCHUNK + c], bounds[s * NCHUNK + c + 1]
            n = int(s1 - s0)
            p0 = t * 128
            p1 = min(p0 + 128, n)
            if p1 <= p0:
                continue
            dseg = ds[s0 + p0:s0 + p1]
            m = (dseg // BLK) == b
            col_s = seg_np[p]
            col_r = rv_np[p]
            idxs = np.nonzero(m)[0]
            col_s[idxs] = (dseg[m] - b * BLK).astype(np.float32)
            col_r[idxs] = rv[dseg[m]]

    # wrap idx into per-call int16 layout
    idx_out = []
    for k in range(P):
        flat = idx_cores[k]
        parts = []
        for (s, c, loc0, cols, _off) in calls:
            base = (sup_col0[s] + loc0) * 128
            parts.append(_wrap16(flat[base:base + cols * 128].astype(np.int16)))
        idx_out.append(np.concatenate(parts, axis=1))
        assert idx_out[-1].shape == (128, IDX_COLS)

    seg_out = [np.ascontiguousarray(s.T) for s in seg_cores]   # [128, NPAIR] fp32
    rv_out = [np.ascontiguousarray(r.T) for r in rv_cores]
    segb_out = [s.astype(NPBF16) for s in seg_out]             # [128, NPAIR] bf16
    rvn_out = []                                               # [128, NB] fp32
    for k in range(P):
        rvn_out.append(np.ascontiguousarray(
            cores[k]["rv"].reshape(NB, BLK).T))

    return dict(calls=calls, pairs=pairs, sup_col0=sup_col0, sup_cols=sup_cols,
                MAXSUPC=MAXSUPC, NPAIR=NPAIR, IDX_COLS=IDX_COLS,
                idx=idx_out, seg=seg_out, rv=rv_out, segb=segb_out,
                rvn=rvn_out)


def _build(meta, mode="full", rep=1):
    calls = meta["calls"]
    pairs = meta["pairs"]
    sup_col0 = meta["sup_col0"]
    MAXSUPC = meta["MAXSUPC"]
    NPAIR = meta["NPAIR"]
    IDX_COLS = meta["IDX_COLS"]

    nc = bacc.Bacc("TRN2", target_bir_lowering=False, debug=False,
                   num_devices=P, num_swdge_queues=4)
    dt = mybir.dt
    xfull_d = nc.dram_tensor("xfull", [NG, F_IN], BF16, kind="ExternalInput")
    xt_d = nc.dram_tensor("xt", [F_IN, NL], BF16, kind="ExternalInput")
    idx_d = nc.dram_tensor("idx", [128, IDX_COLS], dt.int16, kind="ExternalInput")
    seg_d = nc.dram_tensor("seg", [128, NPAIR], dt.float32, kind="ExternalInput")
    rv_d = nc.dram_tensor("rv", [128, NPAIR], dt.float32, kind="ExternalInput")
    iota_d = nc.dram_tensor("iota", [128, BLK], BF16, kind="ExternalInput")
    wl1_d = nc.dram_tensor("W_l1", [F_IN, F_OUT], BF16, kind="ExternalInput")
    wr1_d = nc.dram_tensor("W_r1", [F_IN, F_OUT], BF16, kind="ExternalInput")
    b1_d = nc.dram_tensor("b1", [1, F_OUT], BF16, kind="ExternalInput")
    wl2_d = nc.dram_tensor("W_l2", [F_OUT, F_OUT], BF16, kind="ExternalInput")
    wr2_d = nc.dram_tensor("W_r2", [F_OUT, F_OUT], BF16, kind="ExternalInput")
    b2_d = nc.dram_tensor("b2", [1, F_OUT], BF16, kind="ExternalInput")
    out_d = nc.dram_tensor("out", [F_OUT, NL], dt.float32, kind="ExternalOutput")

    h1pad_d = nc.dram_tensor("h1pad", [NL, F_IN], BF16)
    h1full_d = nc.dram_tensor("h1full", [NG, F_IN], BF16, addr_space="Shared")

    with tile.TileContext(nc) as tc:
        acc_bufs = 4 if mode in ("l1r",) else 2
        with (
            tc.tile_pool(name="const", bufs=1) as constp,
            tc.tile_pool(name="indp", bufs=8) as indp,
            tc.tile_pool(name="op", bufs=4) as op,
            tc.tile_pool(name="ps_acc", bufs=acc_bufs, space="PSUM") as ps_acc,
            tc.tile_pool(name="ps_t", bufs=2, space="PSUM") as ps_t,
            tc.tile_pool(name="ps_o", bufs=2, space="PSUM") as ps_o,
        ):
            iota_t = constp.tile([128, BLK], BF16)
            nc.sync.dma_start(iota_t[:], iota_d[:])
            seg_t = constp.tile([128, NPAIR], dt.float32)
            nc.sync.dma_start(seg_t[:], seg_d[:])
            rv_t = constp.tile([128, NPAIR], dt.float32)
            nc.sync.dma_start(rv_t[:], rv_d[:])
            idx_t = constp.tile([128, IDX_COLS], dt.int16)
            nc.sync.dma_start(idx_t[:], idx_d[:])
            xt_t = constp.tile([F_IN, NL], BF16)
            nc.sync.dma_start(xt_t[:], xt_d[:])
            wl1_t = constp.tile([F_IN, F_OUT], BF16)
            nc.sync.dma_start(wl1_t[:], wl1_d[:])
            wr1_t = constp.tile([F_IN, F_OUT], BF16)
            nc.sync.dma_start(wr1_t[:], wr1_d[:])
            wl2_t = constp.tile([F_OUT, F_OUT], BF16)
            nc.sync.dma_start(wl2_t[:], wl2_d[:])
            wr2_t = constp.tile([F_OUT, F_OUT], BF16)
            nc.sync.dma_start(wr2_t[:], wr2_d[:])
            b1_t = constp.tile([1, F_OUT], BF16)
            nc.sync.dma_start(b1_t[:], b1_d[:])
            b2_t = constp.tile([1, F_OUT], BF16)
            nc.sync.dma_start(b2_t[:], b2_d[:])
            ones_t = constp.tile([1, BLK], BF16)
            nc.vector.memset(ones_t[:], 1.0)
            from concourse.masks import make_identity
            id_t = constp.tile([F_OUT, F_OUT], BF16)
            make_identity(nc, id_t[:])
            h1T_t = constp.tile([F_OUT, NL], BF16)

            stage_a = constp.tile([128, MAXSUPC * F_IN], BF16)
            stage_b = constp.tile([128, MAXSUPC * F_IN], BF16)
            stage = [stage_a, stage_b]
            nc.gpsimd.memset(stage[0][:], 0.0)
            nc.gpsimd.memset(stage[1][:], 0.0)
            if mode in ("gser", "g1s"):
                stage_c = constp.tile([128, MAXSUPC * F_IN], BF16)
                dummy = [stage_c, stage_c]

            qn = [0]
            ident = mybir.ActivationFunctionType

            def gathers(s, table, into=None):
                buf = (into or stage)[s % 2]
                for (ss, c, loc0, cols, ioff) in calls:
                    if ss != s:
                        continue
                    nc.gpsimd.dma_gather(
                        out_ap=buf[:, loc0 * F_IN:(loc0 + cols) * F_IN]
                            .rearrange("p (c f) -> p c f", f=F_IN),
                        in_ap=table[c * CHUNK:min((c + 1) * CHUNK, NG), :],
                        idxs_ap=idx_t[:, ioff:ioff + cols * 8],
                        num_idxs=cols * 128, num_idxs_reg=cols * 128,
                        elem_size=F_IN, single_packet=True,
                        queue_num=qn[0] % 4)
                    qn[0] += 1

            def layer(li, table, FW, wl_t, wr_t, bias_t, selfT, out_sb):
                for s in range(NSUP):
                    if s == 0:
                        gathers(s, table)
                    if s + 1 < NSUP:
                        gathers(s + 1, table)
                    buf = stage[s % 2]
                    for b in range(s * SBK, (s + 1) * SBK):
                        pl = pairs[b]
                        acc = ps_acc.tile([FW, BLK], dt.float32, tag="acc")
                        for j, (loc, pcol) in enumerate(pl):
                            ind = indp.tile([128, BLK], BF16, tag="ind")
                            nc.vector.tensor_scalar(
                                out=ind[:], in0=iota_t[:],
                                scalar1=seg_t[:, pcol:pcol + 1],
                                scalar2=rv_t[:, pcol:pcol + 1],
                                op0=mybir.AluOpType.is_equal,
                                op1=mybir.AluOpType.mult)
                            nc.tensor.matmul(
                                acc[:],
                                lhsT=buf[:, loc * F_IN:loc * F_IN + FW],
                                rhs=ind[:],
                                start=(j == 0), stop=(j == len(pl) - 1))
                        meanT = op.tile([FW, BLK], BF16, tag="meanT")
                        nc.scalar.activation(out=meanT[:], in_=acc[:],
                                             func=ident.Copy)
                        o_ps = ps_o.tile([F_OUT, BLK], dt.float32, tag="ops")
                        nc.tensor.matmul(o_ps[:], lhsT=wl_t[:], rhs=meanT[:],
                                         start=True, stop=False)
                        nc.tensor.matmul(o_ps[:], lhsT=wr_t[:],
                                         rhs=selfT[:, b * BLK:(b + 1) * BLK],
                                         start=False, stop=False)
                        nc.tensor.matmul(o_ps[:], lhsT=bias_t[:1, :],
                                         rhs=ones_t[:1, :],
                                         start=False, stop=True)
                        if li == 1:
                            # h1T block (bf16, relu) kept in SBUF for L2 self
                            nc.scalar.activation(
                                out=out_sb[:, b * BLK:(b + 1) * BLK],
                                in_=o_ps[:], func=ident.Relu)
                            # node-major bf16 copy for the gather table
                            tr = ps_t.tile([BLK, F_OUT], BF16, tag="tr")
                            nc.tensor.transpose(
                                out=tr[:],
                                in_=out_sb[:, b * BLK:(b + 1) * BLK],
                                identity=id_t[:])
                            h1n = op.tile([BLK, F_OUT], BF16, tag="h1n")
                            nc.scalar.activation(out=h1n[:], in_=tr[:],
                                                 func=ident.Copy)
                            nc.sync.dma_start(
                                h1pad_d[b * BLK:(b + 1) * BLK, :F_OUT], h1n[:])
                        else:
                            ob = op.tile([F_OUT, BLK], dt.float32, tag="ob")
                            nc.scalar.activation(out=ob[:], in_=o_ps[:],
                                                 func=ident.Copy)
                            nc.sync.dma_start(
                                out_d[:, b * BLK:(b + 1) * BLK], ob[:])

            def gathers_only():
                for s in range(NSUP):
                    gathers(s, xfull_d)

            def compute_only():
                # L1 pipeline minus the gather calls (stage holds garbage)
                for s in range(NSUP):
                    for b in range(s * SBK, (s + 1) * SBK):
                        pl = pairs[b]
                        acc = ps_acc.tile([F_IN, BLK], dt.float32, tag="acc")
                        for j, (loc, pcol) in enumerate(pl):
                            ind = indp.tile([128, BLK], BF16, tag="ind")
                            nc.vector.tensor_scalar(
                                out=ind[:], in0=iota_t[:],
                                scalar1=seg_t[:, pcol:pcol + 1],
                                scalar2=rv_t[:, pcol:pcol + 1],
                                op0=mybir.AluOpType.is_equal,
                                op1=mybir.AluOpType.mult)
                            nc.tensor.matmul(
                                acc[:],
                                lhsT=stage[s % 2][:, loc * F_IN:
                                                  loc * F_IN + F_IN],
                                rhs=ind[:],
                                start=(j == 0), stop=(j == len(pl) - 1))
                        meanT = op.tile([F_IN, BLK], BF16, tag="meanT")
                        nc.scalar.activation(out=meanT[:], in_=acc[:],
                                             func=ident.Copy)
                        o_ps = ps_o.tile([F_OUT, BLK], dt.float32, tag="ops")
                        nc.tensor.matmul(o_ps[:], lhsT=wl1_t[:], rhs=meanT[:],
                                         start=True, stop=False)
                        nc.tensor.matmul(o_ps[:], lhsT=wr1_t[:],
                                         rhs=xt_t[:, b * BLK:(b + 1) * BLK],
                                         start=False, stop=False)
                        nc.tensor.matmul(o_ps[:], lhsT=b1_t[:1, :],
                                         rhs=ones_t[:1, :],
                                         start=False, stop=True)
                        nc.scalar.activation(
                            out=h1T_t[:, b * BLK:(b + 1) * BLK],
                            in_=o_ps[:], func=ident.Relu)
                        tr = ps_t.tile([BLK, F_OUT], BF16, tag="tr")
                        nc.tensor.transpose(
                            out=tr[:],
                            in_=h1T_t[:, b * BLK:(b + 1) * BLK],
                            identity=id_t[:])
                        h1n = op.tile([BLK, F_OUT], BF16, tag="h1n")
                        nc.scalar.activation(out=h1n[:], in_=tr[:],
                                             func=ident.Copy)
                        nc.sync.dma_start(
                            h1pad_d[b * BLK:(b + 1) * BLK, :F_OUT], h1n[:])

            def compute_nostore():
                for s in range(NSUP):
                    for b in range(s * SBK, (s + 1) * SBK):
                        pl = pairs[b]
                        acc = ps_acc.tile([F_IN, BLK], dt.float32, tag="acc")
                        for j, (loc, pcol) in enumerate(pl):
                            ind = indp.tile([128, BLK], BF16, tag="ind")
                            nc.vector.tensor_scalar(
                                out=ind[:], in0=iota_t[:],
                                scalar1=seg_t[:, pcol:pcol + 1],
                                scalar2=rv_t[:, pcol:pcol + 1],
                                op0=mybir.AluOpType.is_equal,
                                op1=mybir.AluOpType.mult)
                            nc.tensor.matmul(
                                acc[:],
                                lhsT=stage[s % 2][:, loc * F_IN:
                                                  loc * F_IN + F_IN],
                                rhs=ind[:],
                                start=(j == 0), stop=(j == len(pl) - 1))
                        meanT = op.tile([F_IN, BLK], BF16, tag="meanT")
                        nc.scalar.activation(out=meanT[:], in_=acc[:],
                                             func=ident.Copy)
                        o_ps = ps_o.tile([F_OUT, BLK], dt.float32, tag="ops")
                        nc.tensor.matmul(o_ps[:], lhsT=wl1_t[:], rhs=meanT[:],
                                         start=True, stop=False)
                        nc.tensor.matmul(o_ps[:], lhsT=wr1_t[:],
                                         rhs=xt_t[:, b * BLK:(b + 1) * BLK],
                                         start=False, stop=False)
                        nc.tensor.matmul(o_ps[:], lhsT=b1_t[:1, :],
                                         rhs=ones_t[:1, :],
                                         start=False, stop=True)
                        nc.scalar.activation(
                            out=h1T_t[:, b * BLK:(b + 1) * BLK],
                            in_=o_ps[:], func=ident.Relu)

            def l1_pairs_only():
                # gathers + indicator + pair matmuls + acc drain, no tails
                for s in range(NSUP):
                    if s == 0:
                        gathers(s, xfull_d)
                    if s + 1 < NSUP:
                        gathers(s + 1, xfull_d)
                    buf = stage[s % 2]
                    for b in range(s * SBK, (s + 1) * SBK):
                        pl = pairs[b]
                        acc = ps_acc.tile([F_IN, BLK], dt.float32, tag="acc")
                        for j, (loc, pcol) in enumerate(pl):
                            ind = indp.tile([128, BLK], BF16, tag="ind")
                            nc.vector.tensor_scalar(
                                out=ind[:], in0=iota_t[:],
                                scalar1=seg_t[:, pcol:pcol + 1],
                                scalar2=rv_t[:, pcol:pcol + 1],
                                op0=mybir.AluOpType.is_equal,
                                op1=mybir.AluOpType.mult)
                            nc.tensor.matmul(
                                acc[:],
                                lhsT=buf[:, loc * F_IN:loc * F_IN + F_IN],
                                rhs=ind[:],
                                start=(j == 0), stop=(j == len(pl) - 1))
                        meanT = op.tile([F_IN, BLK], BF16, tag="meanT")
                        nc.scalar.activation(out=meanT[:], in_=acc[:],
                                             func=ident.Copy)

            def layer_restr(li, table, FW, wl_t, wr_t, bias_t, selfT,
                            out_sb):
                # all 7 pair-accumulations first, tails after
                for s in range(NSUP):
                    if s == 0:
                        gathers(s, table)
                    if s + 1 < NSUP:
                        gathers(s + 1, table)
                    buf = stage[s % 2]
                    accs = []
                    for b in range(s * SBK, (s + 1) * SBK):
                        pl = pairs[b]
                        acc = ps_acc.tile([FW, BLK], dt.float32, tag="acc")
                        accs.append(acc)
                        for j, (loc, pcol) in enumerate(pl):
                            ind = indp.tile([128, BLK], BF16, tag="ind")
                            nc.vector.tensor_scalar(
                                out=ind[:], in0=iota_t[:],
                                scalar1=seg_t[:, pcol:pcol + 1],
                                scalar2=rv_t[:, pcol:pcol + 1],
                                op0=mybir.AluOpType.is_equal,
                                op1=mybir.AluOpType.mult)
                            nc.tensor.matmul(
                                acc[:],
                                lhsT=buf[:, loc * F_IN:loc * F_IN + FW],
                                rhs=ind[:],
                                start=(j == 0), stop=(j == len(pl) - 1))
                    for bi, b in enumerate(range(s * SBK, (s + 1) * SBK)):
                        acc = accs[bi]
                        meanT = op.tile([FW, BLK], BF16, tag="meanT")
                        nc.scalar.activation(out=meanT[:], in_=acc[:],
                                             func=ident.Copy)
                        o_ps = ps_o.tile([F_OUT, BLK], dt.float32, tag="ops")
                        nc.tensor.matmul(o_ps[:], lhsT=wl_t[:], rhs=meanT[:],
                                         start=True, stop=False)
                        nc.tensor.matmul(o_ps[:], lhsT=wr_t[:],
                                         rhs=selfT[:, b * BLK:(b + 1) * BLK],
                                         start=False, stop=False)
                        nc.tensor.matmul(o_ps[:], lhsT=bias_t[:1, :],
                                         rhs=ones_t[:1, :],
                                         start=False, stop=True)
                        if li == 1:
                            nc.scalar.activation(
                                out=out_sb[:, b * BLK:(b + 1) * BLK],
                                in_=o_ps[:], func=ident.Relu)
                            tr = ps_t.tile([BLK, F_OUT], BF16, tag="tr")
                            nc.tensor.transpose(
                                out=tr[:],
                                in_=out_sb[:, b * BLK:(b + 1) * BLK],
                                identity=id_t[:])
                            h1n = op.tile([BLK, F_OUT], BF16, tag="h1n")
                            nc.scalar.activation(out=h1n[:], in_=tr[:],
                                                 func=ident.Copy)
                            nc.sync.dma_start(
                                h1pad_d[b * BLK:(b + 1) * BLK, :F_OUT],
                                h1n[:])
                        else:
                            ob = op.tile([F_OUT, BLK], dt.float32, tag="ob")
                            nc.scalar.activation(out=ob[:], in_=o_ps[:],
                                                 func=ident.Copy)
                            nc.sync.dma_start(
                                out_d[:, b * BLK:(b + 1) * BLK], ob[:])

            for _r in range(rep):
                if mode == "g1":
                    gathers_only()
                    continue
                if mode == "l1p":
                    l1_pairs_only()
                    continue
                if mode == "l1w":
                    # gathers + ALL pair matmuls, but const rhs (no DVE)
                    for s in range(NSUP):
                        if s == 0:
                            gathers(s, xfull_d)
                        if s + 1 < NSUP:
                            gathers(s + 1, xfull_d)
                        buf = stage[s % 2]
                        for b in range(s * SBK, (s + 1) * SBK):
                            pl = pairs[b]
                            acc = ps_acc.tile([F_IN, BLK], dt.float32,
                                              tag="acc")
                            for j, (loc, pcol) in enumerate(pl):
                                nc.tensor.matmul(
                                    acc[:],
                                    lhsT=buf[:, loc * F_IN:
                                             loc * F_IN + F_IN],
                                    rhs=iota_t[:],
                                    start=(j == 0), stop=(j == len(pl) - 1))
                            meanT = op.tile([F_IN, BLK], BF16, tag="meanT")
                            nc.scalar.activation(out=meanT[:], in_=acc[:],
                                                 func=ident.Copy)
                    continue
                if mode == "l1t":
                    # gathers + one consumer matmul per call (forces drain,
                    # minimal consumer instruction count)
                    for s in range(NSUP):
                        if s == 0:
                            gathers(s, xfull_d)
                        if s + 1 < NSUP:
                            gathers(s + 1, xfull_d)
                        buf = stage[s % 2]
                        acc = ps_acc.tile([F_IN, BLK], dt.float32, tag="acc")
                        scalls = [c for c in calls if c[0] == s]
                        for j, (ss, c, loc0, cols, ioff) in enumerate(scalls):
                            nc.tensor.matmul(
                                acc[:],
                                lhsT=buf[:, loc0 * F_IN:loc0 * F_IN + F_IN],
                                rhs=iota_t[:],
                                start=(j == 0), stop=(j == len(scalls) - 1))
                        meanT = op.tile([F_IN, BLK], BF16, tag="meanT")
                        nc.scalar.activation(out=meanT[:], in_=acc[:],
                                             func=ident.Copy)
                    continue
                if mode == "g1s":
                    for s in range(NSUP):
                        gathers(s, xfull_d, into=dummy)
                    continue
                if mode == "gser":
                    # identical gather stream into dummy bufs + identical
                    # compute on (never-written) real bufs: no data deps
                    for s in range(NSUP):
                        gathers(s, xfull_d, into=dummy)
                    compute_only()
                    continue
                if mode == "l1r":
                    layer_restr(1, xfull_d, F_IN, wl1_t, wr1_t, b1_t,
                                xt_t, h1T_t)
                    continue
                if mode == "c1":
                    compute_only()
                    continue
                if mode == "c1ns":
                    compute_nostore()
                    continue
                if mode == "agonly":
                    nc.gpsimd.collective_compute(
                        "AllGather", mybir.AluOpType.bypass,
                        replica_groups=[list(range(P))],
                        ins=[h1pad_d[:]], outs=[h1full_d[:]])
                    continue
                layer(1, xfull_d, F_IN, wl1_t, wr1_t, b1_t, xt_t, h1T_t)
                if mode == "l1":
                    nc.sync.dma_start(
                        out_d.bitcast(BF16)[:, :NL], h1T_t[:])
                    continue
                nc.gpsimd.collective_compute(
                    "AllGather", mybir.AluOpType.bypass,
                    replica_groups=[list(range(P))],
                    ins=[h1pad_d[:]], outs=[h1full_d[:]])
                if mode == "l1+ag":
                    nc.sync.dma_start(
                        out_d.bitcast(BF16)[:, :NL], h1T_t[:])
                    continue
                layer(2, h1full_d, F_OUT, wl2_t, wr2_t, b2_t, h1T_t, None)
            if mode in ("g1", "agonly", "l1p", "g1s", "l1t", "l1w"):
                nc.sync.dma_start(
                    out_d.bitcast(BF16)[:, :NL], xt_t[:F_OUT, :])
            elif mode in ("c1", "c1ns", "l1r", "gser"):
                nc.sync.dma_start(
                    out_d.bitcast(BF16)[:, :NL], h1T_t[:])

    nc.finalize()
    return nc


ICH = 64   # pairs per bulk-indicator build


def _build3(meta, rep=1, mode="full"):
    """v3: bulk indicator builds (tensor_tensor is_equal over broadcast APs)
    + node-major accumulation acc[dst,F] = ind01^T @ stage with 1/deg as a
    per-partition Act scale at PSUM drain. ~40 DVE instructions per layer
    instead of ~2200 (per-pair tensor_scalar interleaved with SWDGE gathers
    measured ~450ns each of sem/dispatch poison; bulk builds sidestep it).
    """
    calls = meta["calls"]
    pairs = meta["pairs"]
    MAXSUPC = meta["MAXSUPC"]
    NPAIR = meta["NPAIR"]
    IDX_COLS = meta["IDX_COLS"]

    nc = bacc.Bacc("TRN2", target_bir_lowering=False, debug=False,
                   num_devices=P, num_swdge_queues=4)
    dt = mybir.dt
    xfull_d = nc.dram_tensor("xfull", [NG, F_IN], BF16, kind="ExternalInput")
    xt_d = nc.dram_tensor("xt", [F_IN, NL], BF16, kind="ExternalInput")
    idx_d = nc.dram_tensor("idx", [128, IDX_COLS], dt.int16,
                           kind="ExternalInput")
    seg_d = nc.dram_tensor("seg", [128, NPAIR], BF16, kind="ExternalInput")
    rvn_d = nc.dram_tensor("rvn", [128, NB], dt.float32,
                           kind="ExternalInput")
    iota_d = nc.dram_tensor("iota", [128, BLK], BF16, kind="ExternalInput")
    wl1_d = nc.dram_tensor("W_l1", [F_IN, F_OUT], BF16, kind="ExternalInput")
    wr1_d = nc.dram_tensor("W_r1", [F_IN, F_OUT], BF16, kind="ExternalInput")
    b1_d = nc.dram_tensor("b1", [1, F_OUT], BF16, kind="ExternalInput")
    wl2_d = nc.dram_tensor("W_l2", [F_OUT, F_OUT], BF16,
                           kind="ExternalInput")
    wr2_d = nc.dram_tensor("W_r2", [F_OUT, F_OUT], BF16,
                           kind="ExternalInput")
    b2_d = nc.dram_tensor("b2", [1, F_OUT], BF16, kind="ExternalInput")
    out_d = nc.dram_tensor("out", [NL, F_OUT], dt.float32,
                           kind="ExternalOutput")

    h1pad_d = nc.dram_tensor("h1pad", [NL, F_IN], BF16)
    h1full_d = nc.dram_tensor("h1full", [NG, F_IN], BF16, addr_space="Shared")

    # super s covers pair range [srange[s], srange[s+1])
    srange = [pairs[s * SBK][0][1] for s in range(NSUP)] + [NPAIR]

    with tile.TileContext(nc) as tc:
        with (
            tc.tile_pool(name="const", bufs=1) as constp,
            tc.tile_pool(name="indall", bufs=3) as indall,
            tc.tile_pool(name="op", bufs=4) as op,
            tc.tile_pool(name="ps_acc", bufs=2, space="PSUM") as ps_acc,
            tc.tile_pool(name="ps_t", bufs=2, space="PSUM") as ps_t,
            tc.tile_pool(name="ps_o", bufs=2, space="PSUM") as ps_o,
        ):
            iota_t = constp.tile([128, BLK], BF16)
            nc.sync.dma_start(iota_t[:], iota_d[:])
            seg_t = constp.tile([128, NPAIR], BF16)
            nc.sync.dma_start(seg_t[:], seg_d[:])
            rvn_t = constp.tile([128, NB], dt.float32)
            nc.sync.dma_start(rvn_t[:], rvn_d[:])
            idx_t = constp.tile([128, IDX_COLS], dt.int16)
            nc.sync.dma_start(idx_t[:], idx_d[:])
            xt_t = constp.tile([F_IN, NL], BF16)
            nc.sync.dma_start(xt_t[:], xt_d[:])
            wl1_t = constp.tile([F_IN, F_OUT], BF16)
            nc.sync.dma_start(wl1_t[:], wl1_d[:])
            wr1_t = constp.tile([F_IN, F_OUT], BF16)
            nc.sync.dma_start(wr1_t[:], wr1_d[:])
            wl2_t = constp.tile([F_OUT, F_OUT], BF16)
            nc.sync.dma_start(wl2_t[:], wl2_d[:])
            wr2_t = constp.tile([F_OUT, F_OUT], BF16)
            nc.sync.dma_start(wr2_t[:], wr2_d[:])
            b1_t = constp.tile([1, F_OUT], BF16)
            nc.sync.dma_start(b1_t[:], b1_d[:])
            b2_t = constp.tile([1, F_OUT], BF16)
            nc.sync.dma_start(b2_t[:], b2_d[:])
            ones_t = constp.tile([1, BLK], BF16)
            nc.vector.memset(ones_t[:], 1.0)
            from concourse.masks import make_identity
            id128_t = constp.tile([128, 128], BF16)
            make_identity(nc, id128_t[:])
            h1T_t = constp.tile([F_OUT, NL], BF16)

            stage_a = constp.tile([128, MAXSUPC * F_IN], BF16)
            stage_b = constp.tile([128, MAXSUPC * F_IN], BF16)
            stage = [stage_a, stage_b]
            nc.gpsimd.memset(stage[0][:], 0.0)
            nc.gpsimd.memset(stage[1][:], 0.0)

            qn = [0]
            ident = mybir.ActivationFunctionType

            def gathers(s, table):
                buf = stage[s % 2]
                for (ss, c, loc0, cols, ioff) in calls:
                    if ss != s:
                        continue
                    nc.gpsimd.dma_gather(
                        out_ap=buf[:, loc0 * F_IN:(loc0 + cols) * F_IN]
                            .rearrange("p (c f) -> p c f", f=F_IN),
                        in_ap=table[c * CHUNK:min((c + 1) * CHUNK, NG), :],
                        idxs_ap=idx_t[:, ioff:ioff + cols * 8],
                        num_idxs=cols * 128, num_idxs_reg=cols * 128,
                        elem_size=F_IN, single_packet=True,
                        queue_num=qn[0] % 4)
                    qn[0] += 1

            def ind_builds(s):
                # bulk 0/1 indicators for all pairs of super s
                plo, phi = srange[s], srange[s + 1]
                cmap = {}
                for c0 in range(plo, phi, ICH):
                    np_ = min(ICH, phi - c0)
                    ich = indall.tile([128, ICH * 128], BF16, tag="ich")
                    nc.vector.tensor_tensor(
                        out=ich[:, :np_ * 128]
                            .rearrange("p (n j) -> p n j", j=128),
                        in0=seg_t[:, c0:c0 + np_].unsqueeze(2)
                            .broadcast_to([128, np_, 128]),
                        in1=iota_t[:].unsqueeze(1)
                            .broadcast_to([128, np_, BLK]),
                        op=mybir.AluOpType.is_equal)
                    for p in range(c0, c0 + np_):
                        cmap[p] = (ich, p - c0)
                return cmap

            def layer(li, table, FW, wl_t, wr_t, bias_t, selfT,
                      ag_hook=None):
                for s in range(NSUP):
                    if s == 0:
                        gathers(s, table)
                    if s + 1 < NSUP:
                        gathers(s + 1, table)
                    cmap = ind_builds(s)
                    buf = stage[s % 2]
                    for b in range(s * SBK, (s + 1) * SBK):
                        pl = pairs[b]
                        acc = ps_acc.tile([BLK, FW], dt.float32, tag="acc")
                        for j, (loc, pcol) in enumerate(pl):
                            it, off = cmap[pcol]
                            nc.tensor.matmul(
                                acc[:],
                                lhsT=it[:, off * 128:(off + 1) * 128],
                                rhs=buf[:, loc * F_IN:loc * F_IN + FW],
                                start=(j == 0), stop=(j == len(pl) - 1))
                        meanN = op.tile([BLK, FW], BF16, tag="meanN")
                        nc.scalar.activation(out=meanN[:], in_=acc[:],
                                             func=ident.Copy,
                                             scale=rvn_t[:, b:b + 1])
                        trm = ps_t.tile([FW, BLK], BF16, tag="trm")
                        nc.tensor.transpose(out=trm[:], in_=meanN[:],
                                            identity=id128_t[:])
                        meanT = op.tile([FW, BLK], BF16, tag="meanT")
                        nc.scalar.activation(out=meanT[:], in_=trm[:],
                                             func=ident.Copy)
                        o2 = ps_o.tile([BLK, F_OUT], dt.float32, tag="o2")
                        nc.tensor.matmul(o2[:], lhsT=meanT[:], rhs=wl_t[:],
                                         start=True, stop=False)
                        nc.tensor.matmul(o2[:],
                                         lhsT=selfT[:, b * BLK:(b + 1) * BLK],
                                         rhs=wr_t[:],
                                         start=False, stop=False)
                        nc.tensor.matmul(o2[:], lhsT=ones_t[:1, :],
                                         rhs=bias_t[:1, :],
                                         start=False, stop=True)
                        if li == 1:
                            h1n = op.tile([BLK, F_OUT], BF16, tag="h1n")
                            nc.scalar.activation(out=h1n[:], in_=o2[:],
                                                 func=ident.Relu)
                            nc.sync.dma_start(
                                h1pad_d[b * BLK:(b + 1) * BLK, :F_OUT],
                                h1n[:])
                            trh = ps_t.tile([F_OUT, BLK], BF16, tag="trh")
                            nc.tensor.transpose(out=trh[:], in_=h1n[:],
                                                identity=id128_t[:])
                            nc.scalar.activation(
                                out=h1T_t[:, b * BLK:(b + 1) * BLK],
                                in_=trh[:], func=ident.Copy)
                        else:
                            ob = op.tile([BLK, F_OUT], dt.float32, tag="ob")
                            nc.scalar.activation(out=ob[:], in_=o2[:],
                                                 func=ident.Copy)
                            nc.sync.dma_start(
                                out_d[b * BLK:(b + 1) * BLK, :], ob[:])
                    if ag_hook is not None:
                        ag_hook(s)

            def ag_group(g):
                a = g * GRP_ROWS
                nc.gpsimd.collective_compute(
                    "AllGather", mybir.AluOpType.bypass,
                    replica_groups=[list(range(P))],
                    ins=[h1pad_d[a:a + GSZ[g], :]],
                    outs=[h1full_d[GBASE[g]:GBASE[g] + P * GSZ[g], :]])

            for _r in range(rep):
                bounds = list(range(AGS, NSUP, AGS)) + [NSUP]

                def hook(s):
                    if s + 1 in bounds:
                        ag_group(bounds.index(s + 1))

                layer(1, xfull_d, F_IN, wl1_t, wr1_t, b1_t, xt_t,
                      ag_hook=hook)
                layer(2, h1full_d, F_OUT, wl2_t, wr2_t, b2_t, h1T_t)

    nc.finalize()
    return nc


def _make_inputs3(x, W_l1, W_r1, b1, W_l2, W_r2, b2, meta):
    x = np.asarray(x, dtype=np.float32)
    x_full = np.zeros((NG, F_IN), dtype=np.float32)
    slots = np.arange(NREAL)
    for k in range(P):
        x_full[_gmap(k, slots)] = x[k * NREAL:(k + 1) * NREAL]
    x_full_bf = x_full.astype(NPBF16)
    xpad = np.zeros((P, NL, F_IN), dtype=np.float32)
    for k in range(P):
        xpad[k, :NREAL] = x[k * NREAL:(k + 1) * NREAL]
    xpad_bf = xpad.astype(NPBF16)
    iota = np.broadcast_to(np.arange(BLK, dtype=np.float32),
                           (128, BLK)).astype(NPBF16).copy()
    in_maps = []
    for k in range(P):
        in_maps.append({
            "xfull": x_full_bf,
            "xt": np.ascontiguousarray(xpad_bf[k].T),
            "idx": meta["idx"][k],
            "seg": meta["segb"][k],
            "rvn": meta["rvn"][k],
            "iota": iota,
            "W_l1": np.asarray(W_l1, np.float32).astype(NPBF16),
            "W_r1": np.asarray(W_r1, np.float32).astype(NPBF16),
            "b1": np.asarray(b1, np.float32).reshape(1, F_OUT).astype(NPBF16),
            "W_l2": np.asarray(W_l2, np.float32).astype(NPBF16),
            "W_r2": np.asarray(W_r2, np.float32).astype(NPBF16),
            "b2": np.asarray(b2, np.float32).reshape(1, F_OUT).astype(NPBF16),
        })
    return in_maps


def _make_inputs(x, W_l1, W_r1, b1, W_l2, W_r2, b2, meta):
    x = np.asarray(x, dtype=np.float32)
    x_full = np.zeros((NG, F_IN), dtype=np.float32)
    for k in range(P):
        x_full[k * NL:k * NL + NREAL] = x[k * NREAL:(k + 1) * NREAL]
    x_full_bf = x_full.astype(NPBF16)
    iota = np.broadcast_to(np.arange(BLK, dtype=np.float32),
                           (128, BLK)).astype(NPBF16).copy()
    in_maps = []
    for k in range(P):
        in_maps.append({
            "xfull": x_full_bf,
            "xt": np.ascontiguousarray(x_full_bf[k * NL:(k + 1) * NL].T),
            "idx": meta["idx"][k],
            "seg": meta["seg"][k],
            "rv": meta["rv"][k],
            "iota": iota,
            "W_l1": np.asarray(W_l1, np.float32).astype(NPBF16),
            "W_r1": np.asarray(W_r1, np.float32).astype(NPBF16),
            "b1": np.asarray(b1, np.float32).reshape(1, F_OUT).astype(NPBF16),
            "W_l2": np.asarray(W_l2, np.float32).astype(NPBF16),
            "W_r2": np.asarray(W_r2, np.float32).astype(NPBF16),
            "b2": np.asarray(b2, np.float32).reshape(1, F_OUT).astype(NPBF16),
        })
    return in_maps


def kernel(x, edge_index, W_l1, W_r1, b1, W_l2, W_r2, b2):
    meta = _preprocess(np.asarray(edge_index))
    in_maps = _make_inputs3(x, W_l1, W_r1, b1, W_l2, W_r2, b2, meta)
    nc = _build3(meta)
    res = run_bass_kernel_spmd(nc, in_maps, core_ids=list(range(P)))
    out = np.concatenate(
        [res.results[k]["out"][:NREAL] for k in range(P)], axis=0)
    return out.astype(np.float32)


if __name__ == "__main__":
    rng = np.random.default_rng(0)
    x = rng.normal(size=(N_NODES, F_IN)).astype(np.float32)
    ei = rng.integers(0, N_NODES, size=(2, N_EDGES)).astype(np.int64)
    wl1 = rng.normal(size=(F_IN, F_OUT)).astype(np.float32) / np.sqrt(F_IN)
    wr1 = rng.normal(size=(F_IN, F_OUT)).astype(np.float32) / np.sqrt(F_IN)
    wl2 = rng.normal(size=(F_OUT, F_OUT)).astype(np.float32) / np.sqrt(F_OUT)
    wr2 = rng.normal(size=(F_OUT, F_OUT)).astype(np.float32) / np.sqrt(F_OUT)
    b1 = np.zeros(F_OUT, np.float32)
    b2 = np.zeros(F_OUT, np.float32)
    out = kernel(x, ei, wl1, wr1, b1, wl2, wr2, b2)
    print("out", out.shape, out.dtype, float(np.abs(out).mean()))



# revision 17
# speedup vs baseline: 1.1753x; 1.1753x over previous
"""2-layer GraphSAGE (mean aggregation) on 8 Trainium2 NeuronCores — v2.

Strategy (dst-sharded graph parallel), changes vs v1:
- bf16 data path: x table, staged gathers, indicators, weight matmuls (PE
  1cyc/row vs fp32 4), PSUM accumulate fp32.
- Transposed accumulation: acc[F, dst] = stage[pos, F].T @ ind[pos, dst];
  1/deg folded into the indicator value (tensor_scalar is_equal * rv), so
  PSUM holds mean^T directly -> no per-block scale/transpose chain.
- Superblock gather calls: ~4096 indices/call (vs 512) -> ~8x fewer SWDGE
  fixed overheads on Pool. Trailing pad indices are -1 (trimmed by ucode,
  no descriptors) instead of gathering row 0.
- Both layers share one idx/seg/rv tensor set (same edge structure, both
  tables are 256B-row bf16 [NG, 128]).
- Layer outputs are produced transposed [64, NL]; host untransposes.
"""
import sys
sys.path.insert(0, "/opt/trn_rl_repo")
import numpy as np

import concourse.bass as bass
import concourse.bacc as bacc
import concourse.mybir as mybir
import concourse.tile as tile
from concourse.bass_utils import run_bass_kernel_spmd

N_NODES = 100000
N_EDGES = 1600000
F_IN = 128
F_OUT = 64
P = 8
NREAL = 12500
NL = 12544            # 98 * 128
BLK = 128
NB = NL // BLK        # 98
SBK = 7               # blocks per superblock
NSUP = NB // SBK      # 14
SUPN = SBK * BLK      # 896 dsts per super
CHUNK = 32768
NCHUNK = (P * NL + CHUNK - 1) // CHUNK   # 4
NG = P * NL           # 100352
GCOLS = 8             # max 128-idx cols per gather call (1024 idxs).
                      # Empirical HW limits: 2048-idx calls deadlock the
                      # SWDGE ring (129 descs/engine > 128 in-flight cap);
                      # 1536/1920 also fail (Q7 idx scratch); 1024 is stable.
SENT = 999.0
AGS = 7                              # supers per AllGather group
GRP_ROWS = AGS * SUPN                # 3584
NGRP = (NSUP + AGS - 1) // AGS       # 4
GSZ = [min(NSUP, (g + 1) * AGS) * SUPN - g * GRP_ROWS for g in range(NGRP)]
GBASE = [0] * NGRP                   # global row base of each group
for _g in range(1, NGRP):
    GBASE[_g] = GBASE[_g - 1] + P * GSZ[_g - 1]


def _gmap(core, slot):
    # group-major global table row for (core, local slot)
    g = np.minimum(slot // GRP_ROWS, NGRP - 1)
    gsz = np.asarray(GSZ)[g]
    gbase = np.asarray(GBASE)[g]
    return gbase + core * gsz + (slot - g * GRP_ROWS)

BF16 = mybir.dt.bfloat16
NPBF16 = mybir.dt.np(BF16)


def _wrap16(flat_idx):
    w = flat_idx.reshape(-1, 16).T.copy()
    return np.tile(w, (8, 1))


def _preprocess(edge_index):
    src = np.asarray(edge_index[0], dtype=np.int64)
    dst = np.asarray(edge_index[1], dtype=np.int64)
    dcore = dst // NREAL
    dslot = dst - dcore * NREAL
    score = src // NREAL
    g_src = _gmap(score, src - score * NREAL)

    cores = []           # per core dict: ds, g (sorted), seg boundaries
    for k in range(P):
        sel = dcore == k
        ds = dslot[sel]
        g = g_src[sel]
        ch = g // CHUNK
        sup = ds // SUPN
        order = np.lexsort((g, ds, ch, sup))
        ds, g, ch, sup = ds[order], g[order], ch[order], sup[order]
        code = sup * NCHUNK + ch
        bounds = np.searchsorted(code, np.arange(NSUP * NCHUNK + 1))
        cnt = np.bincount(ds, minlength=NL).astype(np.float64)
        rv = (1.0 / np.maximum(cnt, 1.0)).astype(np.float32)
        cores.append(dict(ds=ds, g=g, bounds=bounds, rv=rv))

    # uniform cols per (sup, chunk)
    ncols = np.zeros((NSUP, NCHUNK), dtype=np.int64)
    for k in range(P):
        b = cores[k]["bounds"]
        n = (b[1:] - b[:-1]).reshape(NSUP, NCHUNK)
        ncols = np.maximum(ncols, (n + 127) // 128)
    seg_col0 = np.zeros((NSUP, NCHUNK), dtype=np.int64)   # global col base
    sup_col0 = np.zeros(NSUP, dtype=np.int64)             # col base within super
    tot = 0
    for s in range(NSUP):
        loc = 0
        for c in range(NCHUNK):
            seg_col0[s, c] = loc          # local to super
            loc += int(ncols[s, c])
        sup_col0[s] = tot
        tot += loc
    sup_cols = [int(ncols[s].sum()) for s in range(NSUP)]
    MAXSUPC = max(sup_cols)

    # calls: (s, c, loc_col0, cols, idx_off) — uniform
    calls = []
    idx_off = 0
    for s in range(NSUP):
        for c in range(NCHUNK):
            nc_ = int(ncols[s, c])
            done = 0
            while done < nc_:
                piece = min(GCOLS, nc_ - done)
                calls.append((s, c, int(seg_col0[s, c]) + done, piece, idx_off))
                idx_off += piece * 8
                done += piece
    IDX_COLS = idx_off

    # block windows (uniform): for each block b, chunk c -> [wlo, whi) local cols
    wins = np.zeros((NB, NCHUNK, 2), dtype=np.int64)
    wins[:, :, 0] = 1 << 60
    for k in range(P):
        ds, bounds = cores[k]["ds"], cores[k]["bounds"]
        for s in range(NSUP):
            for c in range(NCHUNK):
                s0, s1 = bounds[s * NCHUNK + c], bounds[s * NCHUNK + c + 1]
                blkseg = ds[s0:s1] // BLK
                for b in range(s * SBK, (s + 1) * SBK):
                    lo = int(np.searchsorted(blkseg, b))
                    hi = int(np.searchsorted(blkseg, b + 1))
                    if hi > lo:
                        wins[b, c, 0] = min(wins[b, c, 0], lo // 128)
                        wins[b, c, 1] = max(wins[b, c, 1], (hi + 127) // 128)

    # pairs: per block, list of (loc_col, pair_idx)
    pairs = [[] for _ in range(NB)]
    npair = 0
    for b in range(NB):
        s = b // SBK
        for c in range(NCHUNK):
            wlo, whi = wins[b, c]
            if whi <= wlo:
                continue
            for t in range(int(wlo), int(whi)):
                pairs[b].append((int(seg_col0[s, c]) + t, npair))
                npair += 1
    NPAIR = npair

    # per-core tensors
    idx_cores, seg_cores, rv_cores = [], [], []
    for k in range(P):
        ds, g, bounds, rv = (cores[k][x] for x in ("ds", "g", "bounds", "rv"))
        seg_np = np.full((NPAIR, 128), SENT, dtype=np.float32)
        rv_np = np.zeros((NPAIR, 128), dtype=np.float32)
        # pad positions gather row 0 (cheap, finite); sentinel seg zeroes
        # their contribution. Negative (skipped) indices desync the SWDGE
        # ring bookkeeping (decode reserves untrimmed, gen trims) -> hang.
        idx_flat = np.zeros(tot * 128, dtype=np.int64)
        for s in range(NSUP):
            for c in range(NCHUNK):
                s0, s1 = bounds[s * NCHUNK + c], bounds[s * NCHUNK + c + 1]
                n = s1 - s0
                base = (sup_col0[s] + seg_col0[s, c]) * 128
                idx_flat[base:base + n] = g[s0:s1] - c * CHUNK
        idx_cores.append(idx_flat)
        seg_cores.append(seg_np)
        rv_cores.append(rv_np)

    # fill seg/rv per pair (redo with pair indices known)
    pair_list = []   # (b, c, t_local)
    for b in range(NB):
        s = b // SBK
        for c in range(NCHUNK):
            wlo, whi = wins[b, c]
            for t in range(int(wlo), int(whi)):
                pair_list.append((b, c, t))
    assert len(pair_list) == NPAIR
    for k in range(P):
        ds, bounds, rv = (cores[k][x] for x in ("ds", "bounds", "rv"))
        seg_np = seg_cores[k]
        rv_np = rv_cores[k]
        for p, (b, c, t) in enumerate(pair_list):
            s = b // SBK
            s0, s1 = bounds[s * NCHUNK + c], bounds[s * NCHUNK + c + 1]
            n = int(s1 - s0)
            p0 = t * 128
            p1 = min(p0 + 128, n)
            if p1 <= p0:
                continue
            dseg = ds[s0 + p0:s0 + p1]
            m = (dseg // BLK) == b
            col_s = seg_np[p]
            col_r = rv_np[p]
            idxs = np.nonzero(m)[0]
            col_s[idxs] = (dseg[m] - b * BLK).astype(np.float32)
            col_r[idxs] = rv[dseg[m]]

    # wrap idx into per-call int16 layout
    idx_out = []
    for k in range(P):
        flat = idx_cores[k]
        parts = []
        for (s, c, loc0, cols, _off) in calls:
            base = (sup_col0[s] + loc0) * 128
            parts.append(_wrap16(flat[base:base + cols * 128].astype(np.int16)))
        idx_out.append(np.concatenate(parts, axis=1))
        assert idx_out[-1].shape == (128, IDX_COLS)

    seg_out = [np.ascontiguousarray(s.T) for s in seg_cores]   # [128, NPAIR] fp32
    rv_out = [np.ascontiguousarray(r.T) for r in rv_cores]
    segb_out = [s.astype(NPBF16) for s in seg_out]             # [128, NPAIR] bf16
    rvn_out = []                                               # [128, NB] fp32
    for k in range(P):
        rvn_out.append(np.ascontiguousarray(
            cores[k]["rv"].reshape(NB, BLK).T))

    return dict(calls=calls, pairs=pairs, sup_col0=sup_col0, sup_cols=sup_cols,
                MAXSUPC=MAXSUPC, NPAIR=NPAIR, IDX_COLS=IDX_COLS,
                idx=idx_out, seg=seg_out, rv=rv_out, segb=segb_out,
                rvn=rvn_out)


def _build(meta, mode="full", rep=1):
    calls = meta["calls"]
    pairs = meta["pairs"]
    sup_col0 = meta["sup_col0"]
    MAXSUPC = meta["MAXSUPC"]
    NPAIR = meta["NPAIR"]
    IDX_COLS = meta["IDX_COLS"]

    nc = bacc.Bacc("TRN2", target_bir_lowering=False, debug=False,
                   num_devices=P, num_swdge_queues=4)
    dt = mybir.dt
    xfull_d = nc.dram_tensor("xfull", [NG, F_IN], BF16, kind="ExternalInput")
    xt_d = nc.dram_tensor("xt", [F_IN, NL], BF16, kind="ExternalInput")
    idx_d = nc.dram_tensor("idx", [128, IDX_COLS], dt.int16, kind="ExternalInput")
    seg_d = nc.dram_tensor("seg", [128, NPAIR], dt.float32, kind="ExternalInput")
    rv_d = nc.dram_tensor("rv", [128, NPAIR], dt.float32, kind="ExternalInput")
    iota_d = nc.dram_tensor("iota", [128, BLK], BF16, kind="ExternalInput")
    wl1_d = nc.dram_tensor("W_l1", [F_IN, F_OUT], BF16, kind="ExternalInput")
    wr1_d = nc.dram_tensor("W_r1", [F_IN, F_OUT], BF16, kind="ExternalInput")
    b1_d = nc.dram_tensor("b1", [1, F_OUT], BF16, kind="ExternalInput")
    wl2_d = nc.dram_tensor("W_l2", [F_OUT, F_OUT], BF16, kind="ExternalInput")
    wr2_d = nc.dram_tensor("W_r2", [F_OUT, F_OUT], BF16, kind="ExternalInput")
    b2_d = nc.dram_tensor("b2", [1, F_OUT], BF16, kind="ExternalInput")
    out_d = nc.dram_tensor("out", [F_OUT, NL], dt.float32, kind="ExternalOutput")

    h1pad_d = nc.dram_tensor("h1pad", [NL, F_IN], BF16)
    h1full_d = nc.dram_tensor("h1full", [NG, F_IN], BF16, addr_space="Shared")

    with tile.TileContext(nc) as tc:
        acc_bufs = 4 if mode in ("l1r",) else 2
        with (
            tc.tile_pool(name="const", bufs=1) as constp,
            tc.tile_pool(name="indp", bufs=8) as indp,
            tc.tile_pool(name="op", bufs=4) as op,
            tc.tile_pool(name="ps_acc", bufs=acc_bufs, space="PSUM") as ps_acc,
            tc.tile_pool(name="ps_t", bufs=2, space="PSUM") as ps_t,
            tc.tile_pool(name="ps_o", bufs=2, space="PSUM") as ps_o,
        ):
            iota_t = constp.tile([128, BLK], BF16)
            nc.sync.dma_start(iota_t[:], iota_d[:])
            seg_t = constp.tile([128, NPAIR], dt.float32)
            nc.sync.dma_start(seg_t[:], seg_d[:])
            rv_t = constp.tile([128, NPAIR], dt.float32)
            nc.sync.dma_start(rv_t[:], rv_d[:])
            idx_t = constp.tile([128, IDX_COLS], dt.int16)
            nc.sync.dma_start(idx_t[:], idx_d[:])
            xt_t = constp.tile([F_IN, NL], BF16)
            nc.sync.dma_start(xt_t[:], xt_d[:])
            wl1_t = constp.tile([F_IN, F_OUT], BF16)
            nc.sync.dma_start(wl1_t[:], wl1_d[:])
            wr1_t = constp.tile([F_IN, F_OUT], BF16)
            nc.sync.dma_start(wr1_t[:], wr1_d[:])
            wl2_t = constp.tile([F_OUT, F_OUT], BF16)
            nc.sync.dma_start(wl2_t[:], wl2_d[:])
            wr2_t = constp.tile([F_OUT, F_OUT], BF16)
            nc.sync.dma_start(wr2_t[:], wr2_d[:])
            b1_t = constp.tile([1, F_OUT], BF16)
            nc.sync.dma_start(b1_t[:], b1_d[:])
            b2_t = constp.tile([1, F_OUT], BF16)
            nc.sync.dma_start(b2_t[:], b2_d[:])
            ones_t = constp.tile([1, BLK], BF16)
            nc.vector.memset(ones_t[:], 1.0)
            from concourse.masks import make_identity
            id_t = constp.tile([F_OUT, F_OUT], BF16)
            make_identity(nc, id_t[:])
            h1T_t = constp.tile([F_OUT, NL], BF16)

            stage_a = constp.tile([128, MAXSUPC * F_IN], BF16)
            stage_b = constp.tile([128, MAXSUPC * F_IN], BF16)
            stage = [stage_a, stage_b]
            nc.gpsimd.memset(stage[0][:], 0.0)
            nc.gpsimd.memset(stage[1][:], 0.0)
            if mode in ("gser", "g1s"):
                stage_c = constp.tile([128, MAXSUPC * F_IN], BF16)
                dummy = [stage_c, stage_c]

            qn = [0]
            ident = mybir.ActivationFunctionType

            def gathers(s, table, into=None):
                buf = (into or stage)[s % 2]
                for (ss, c, loc0, cols, ioff) in calls:
                    if ss != s:
                        continue
                    nc.gpsimd.dma_gather(
                        out_ap=buf[:, loc0 * F_IN:(loc0 + cols) * F_IN]
                            .rearrange("p (c f) -> p c f", f=F_IN),
                        in_ap=table[c * CHUNK:min((c + 1) * CHUNK, NG), :],
                        idxs_ap=idx_t[:, ioff:ioff + cols * 8],
                        num_idxs=cols * 128, num_idxs_reg=cols * 128,
                        elem_size=F_IN, single_packet=True,
                        queue_num=qn[0] % 4)
                    qn[0] += 1

            def layer(li, table, FW, wl_t, wr_t, bias_t, selfT, out_sb):
                for s in range(NSUP):
                    if s == 0:
                        gathers(s, table)
                    if s + 1 < NSUP:
                        gathers(s + 1, table)
                    buf = stage[s % 2]
                    for b in range(s * SBK, (s + 1) * SBK):
                        pl = pairs[b]
                        acc = ps_acc.tile([FW, BLK], dt.float32, tag="acc")
                        for j, (loc, pcol) in enumerate(pl):
                            ind = indp.tile([128, BLK], BF16, tag="ind")
                            nc.vector.tensor_scalar(
                                out=ind[:], in0=iota_t[:],
                                scalar1=seg_t[:, pcol:pcol + 1],
                                scalar2=rv_t[:, pcol:pcol + 1],
                                op0=mybir.AluOpType.is_equal,
                                op1=mybir.AluOpType.mult)
                            nc.tensor.matmul(
                                acc[:],
                                lhsT=buf[:, loc * F_IN:loc * F_IN + FW],
                                rhs=ind[:],
                                start=(j == 0), stop=(j == len(pl) - 1))
                        meanT = op.tile([FW, BLK], BF16, tag="meanT")
                        nc.scalar.activation(out=meanT[:], in_=acc[:],
                                             func=ident.Copy)
                        o_ps = ps_o.tile([F_OUT, BLK], dt.float32, tag="ops")
                        nc.tensor.matmul(o_ps[:], lhsT=wl_t[:], rhs=meanT[:],
                                         start=True, stop=False)
                        nc.tensor.matmul(o_ps[:], lhsT=wr_t[:],
                                         rhs=selfT[:, b * BLK:(b + 1) * BLK],
                                         start=False, stop=False)
                        nc.tensor.matmul(o_ps[:], lhsT=bias_t[:1, :],
                                         rhs=ones_t[:1, :],
                                         start=False, stop=True)
                        if li == 1:
                            # h1T block (bf16, relu) kept in SBUF for L2 self
                            nc.scalar.activation(
                                out=out_sb[:, b * BLK:(b + 1) * BLK],
                                in_=o_ps[:], func=ident.Relu)
                            # node-major bf16 copy for the gather table
                            tr = ps_t.tile([BLK, F_OUT], BF16, tag="tr")
                            nc.tensor.transpose(
                                out=tr[:],
                                in_=out_sb[:, b * BLK:(b + 1) * BLK],
                                identity=id_t[:])
                            h1n = op.tile([BLK, F_OUT], BF16, tag="h1n")
                            nc.scalar.activation(out=h1n[:], in_=tr[:],
                                                 func=ident.Copy)
                            nc.sync.dma_start(
                                h1pad_d[b * BLK:(b + 1) * BLK, :F_OUT], h1n[:])
                        else:
                            ob = op.tile([F_OUT, BLK], dt.float32, tag="ob")
                            nc.scalar.activation(out=ob[:], in_=o_ps[:],
                                                 func=ident.Copy)
                            nc.sync.dma_start(
                                out_d[:, b * BLK:(b + 1) * BLK], ob[:])

            def gathers_only():
                for s in range(NSUP):
                    gathers(s, xfull_d)

            def compute_only():
                # L1 pipeline minus the gather calls (stage holds garbage)
                for s in range(NSUP):
                    for b in range(s * SBK, (s + 1) * SBK):
                        pl = pairs[b]
                        acc = ps_acc.tile([F_IN, BLK], dt.float32, tag="acc")
                        for j, (loc, pcol) in enumerate(pl):
                            ind = indp.tile([128, BLK], BF16, tag="ind")
                            nc.vector.tensor_scalar(
                                out=ind[:], in0=iota_t[:],
                                scalar1=seg_t[:, pcol:pcol + 1],
                                scalar2=rv_t[:, pcol:pcol + 1],
                                op0=mybir.AluOpType.is_equal,
                                op1=mybir.AluOpType.mult)
                            nc.tensor.matmul(
                                acc[:],
                                lhsT=stage[s % 2][:, loc * F_IN:
                                                  loc * F_IN + F_IN],
                                rhs=ind[:],
                                start=(j == 0), stop=(j == len(pl) - 1))
                        meanT = op.tile([F_IN, BLK], BF16, tag="meanT")
                        nc.scalar.activation(out=meanT[:], in_=acc[:],
                                             func=ident.Copy)
                        o_ps = ps_o.tile([F_OUT, BLK], dt.float32, tag="ops")
                        nc.tensor.matmul(o_ps[:], lhsT=wl1_t[:], rhs=meanT[:],
                                         start=True, stop=False)
                        nc.tensor.matmul(o_ps[:], lhsT=wr1_t[:],
                                         rhs=xt_t[:, b * BLK:(b + 1) * BLK],
                                         start=False, stop=False)
                        nc.tensor.matmul(o_ps[:], lhsT=b1_t[:1, :],
                                         rhs=ones_t[:1, :],
                                         start=False, stop=True)
                        nc.scalar.activation(
                            out=h1T_t[:, b * BLK:(b + 1) * BLK],
                            in_=o_ps[:], func=ident.Relu)
                        tr = ps_t.tile([BLK, F_OUT], BF16, tag="tr")
                        nc.tensor.transpose(
                            out=tr[:],
                            in_=h1T_t[:, b * BLK:(b + 1) * BLK],
                            identity=id_t[:])
                        h1n = op.tile([BLK, F_OUT], BF16, tag="h1n")
                        nc.scalar.activation(out=h1n[:], in_=tr[:],
                                             func=ident.Copy)
                        nc.sync.dma_start(
                            h1pad_d[b * BLK:(b + 1) * BLK, :F_OUT], h1n[:])

            def compute_nostore():
                for s in range(NSUP):
                    for b in range(s * SBK, (s + 1) * SBK):
                        pl = pairs[b]
                        acc = ps_acc.tile([F_IN, BLK], dt.float32, tag="acc")
                        for j, (loc, pcol) in enumerate(pl):
                            ind = indp.tile([128, BLK], BF16, tag="ind")
                            nc.vector.tensor_scalar(
                                out=ind[:], in0=iota_t[:],
                                scalar1=seg_t[:, pcol:pcol + 1],
                                scalar2=rv_t[:, pcol:pcol + 1],
                                op0=mybir.AluOpType.is_equal,
                                op1=mybir.AluOpType.mult)
                            nc.tensor.matmul(
                                acc[:],
                                lhsT=stage[s % 2][:, loc * F_IN:
                                                  loc * F_IN + F_IN],
                                rhs=ind[:],
                                start=(j == 0), stop=(j == len(pl) - 1))
                        meanT = op.tile([F_IN, BLK], BF16, tag="meanT")
                        nc.scalar.activation(out=meanT[:], in_=acc[:],
                                             func=ident.Copy)
                        o_ps = ps_o.tile([F_OUT, BLK], dt.float32, tag="ops")
                        nc.tensor.matmul(o_ps[:], lhsT=wl1_t[:], rhs=meanT[:],
                                         start=True, stop=False)
                        nc.tensor.matmul(o_ps[:], lhsT=wr1_t[:],
                                         rhs=xt_t[:, b * BLK:(b + 1) * BLK],
                                         start=False, stop=False)
                        nc.tensor.matmul(o_ps[:], lhsT=b1_t[:1, :],
                                         rhs=ones_t[:1, :],
                                         start=False, stop=True)
                        nc.scalar.activation(
                            out=h1T_t[:, b * BLK:(b + 1) * BLK],
                            in_=o_ps[:], func=ident.Relu)

            def l1_pairs_only():
                # gathers + indicator + pair matmuls + acc drain, no tails
                for s in range(NSUP):
                    if s == 0:
                        gathers(s, xfull_d)
                    if s + 1 < NSUP:
                        gathers(s + 1, xfull_d)
                    buf = stage[s % 2]
                    for b in range(s * SBK, (s + 1) * SBK):
                        pl = pairs[b]
                        acc = ps_acc.tile([F_IN, BLK], dt.float32, tag="acc")
                        for j, (loc, pcol) in enumerate(pl):
                            ind = indp.tile([128, BLK], BF16, tag="ind")
                            nc.vector.tensor_scalar(
                                out=ind[:], in0=iota_t[:],
                                scalar1=seg_t[:, pcol:pcol + 1],
                                scalar2=rv_t[:, pcol:pcol + 1],
                                op0=mybir.AluOpType.is_equal,
                                op1=mybir.AluOpType.mult)
                            nc.tensor.matmul(
                                acc[:],
                                lhsT=buf[:, loc * F_IN:loc * F_IN + F_IN],
                                rhs=ind[:],
                                start=(j == 0), stop=(j == len(pl) - 1))
                        meanT = op.tile([F_IN, BLK], BF16, tag="meanT")
                        nc.scalar.activation(out=meanT[:], in_=acc[:],
                                             func=ident.Copy)

            def layer_restr(li, table, FW, wl_t, wr_t, bias_t, selfT,
                            out_sb):
                # all 7 pair-accumulations first, tails after
                for s in range(NSUP):
                    if s == 0:
                        gathers(s, table)
                    if s + 1 < NSUP:
                        gathers(s + 1, table)
                    buf = stage[s % 2]
                    accs = []
                    for b in range(s * SBK, (s + 1) * SBK):
                        pl = pairs[b]
                        acc = ps_acc.tile([FW, BLK], dt.float32, tag="acc")
                        accs.append(acc)
                        for j, (loc, pcol) in enumerate(pl):
                            ind = indp.tile([128, BLK], BF16, tag="ind")
                            nc.vector.tensor_scalar(
                                out=ind[:], in0=iota_t[:],
                                scalar1=seg_t[:, pcol:pcol + 1],
                                scalar2=rv_t[:, pcol:pcol + 1],
                                op0=mybir.AluOpType.is_equal,
                                op1=mybir.AluOpType.mult)
                            nc.tensor.matmul(
                                acc[:],
                                lhsT=buf[:, loc * F_IN:loc * F_IN + FW],
                                rhs=ind[:],
                                start=(j == 0), stop=(j == len(pl) - 1))
                    for bi, b in enumerate(range(s * SBK, (s + 1) * SBK)):
                        acc = accs[bi]
                        meanT = op.tile([FW, BLK], BF16, tag="meanT")
                        nc.scalar.activation(out=meanT[:], in_=acc[:],
                                             func=ident.Copy)
                        o_ps = ps_o.tile([F_OUT, BLK], dt.float32, tag="ops")
                        nc.tensor.matmul(o_ps[:], lhsT=wl_t[:], rhs=meanT[:],
                                         start=True, stop=False)
                        nc.tensor.matmul(o_ps[:], lhsT=wr_t[:],
                                         rhs=selfT[:, b * BLK:(b + 1) * BLK],
                                         start=False, stop=False)
                        nc.tensor.matmul(o_ps[:], lhsT=bias_t[:1, :],
                                         rhs=ones_t[:1, :],
                                         start=False, stop=True)
                        if li == 1:
                            nc.scalar.activation(
                                out=out_sb[:, b * BLK:(b + 1) * BLK],
                                in_=o_ps[:], func=ident.Relu)
                            tr = ps_t.tile([BLK, F_OUT], BF16, tag="tr")
                            nc.tensor.transpose(
                                out=tr[:],
                                in_=out_sb[:, b * BLK:(b + 1) * BLK],
                                identity=id_t[:])
                            h1n = op.tile([BLK, F_OUT], BF16, tag="h1n")
                            nc.scalar.activation(out=h1n[:], in_=tr[:],
                                                 func=ident.Copy)
                            nc.sync.dma_start(
                                h1pad_d[b * BLK:(b + 1) * BLK, :F_OUT],
                                h1n[:])
                        else:
                            ob = op.tile([F_OUT, BLK], dt.float32, tag="ob")
                            nc.scalar.activation(out=ob[:], in_=o_ps[:],
                                                 func=ident.Copy)
                            nc.sync.dma_start(
                                out_d[:, b * BLK:(b + 1) * BLK], ob[:])

            for _r in range(rep):
                if mode == "g1":
                    gathers_only()
                    continue
                if mode == "l1p":
                    l1_pairs_only()
                    continue
                if mode == "l1w":
                    # gathers + ALL pair matmuls, but const rhs (no DVE)
                    for s in range(NSUP):
                        if s == 0:
                            gathers(s, xfull_d)
                        if s + 1 < NSUP:
                            gathers(s + 1, xfull_d)
                        buf = stage[s % 2]
                        for b in range(s * SBK, (s + 1) * SBK):
                            pl = pairs[b]
                            acc = ps_acc.tile([F_IN, BLK], dt.float32,
                                              tag="acc")
                            for j, (loc, pcol) in enumerate(pl):
                                nc.tensor.matmul(
                                    acc[:],
                                    lhsT=buf[:, loc * F_IN:
                                             loc * F_IN + F_IN],
                                    rhs=iota_t[:],
                                    start=(j == 0), stop=(j == len(pl) - 1))
                            meanT = op.tile([F_IN, BLK], BF16, tag="meanT")
                            nc.scalar.activation(out=meanT[:], in_=acc[:],
                                                 func=ident.Copy)
                    continue
                if mode == "l1t":
                    # gathers + one consumer matmul per call (forces drain,
                    # minimal consumer instruction count)
                    for s in range(NSUP):
                        if s == 0:
                            gathers(s, xfull_d)
                        if s + 1 < NSUP:
                            gathers(s + 1, xfull_d)
                        buf = stage[s % 2]
                        acc = ps_acc.tile([F_IN, BLK], dt.float32, tag="acc")
                        scalls = [c for c in calls if c[0] == s]
                        for j, (ss, c, loc0, cols, ioff) in enumerate(scalls):
                            nc.tensor.matmul(
                                acc[:],
                                lhsT=buf[:, loc0 * F_IN:loc0 * F_IN + F_IN],
                                rhs=iota_t[:],
                                start=(j == 0), stop=(j == len(scalls) - 1))
                        meanT = op.tile([F_IN, BLK], BF16, tag="meanT")
                        nc.scalar.activation(out=meanT[:], in_=acc[:],
                                             func=ident.Copy)
                    continue
                if mode == "g1s":
                    for s in range(NSUP):
                        gathers(s, xfull_d, into=dummy)
                    continue
                if mode == "gser":
                    # identical gather stream into dummy bufs + identical
                    # compute on (never-written) real bufs: no data deps
                    for s in range(NSUP):
                        gathers(s, xfull_d, into=dummy)
                    compute_only()
                    continue
                if mode == "l1r":
                    layer_restr(1, xfull_d, F_IN, wl1_t, wr1_t, b1_t,
                                xt_t, h1T_t)
                    continue
                if mode == "c1":
                    compute_only()
                    continue
                if mode == "c1ns":
                    compute_nostore()
                    continue
                if mode == "agonly":
                    nc.gpsimd.collective_compute(
                        "AllGather", mybir.AluOpType.bypass,
                        replica_groups=[list(range(P))],
                        ins=[h1pad_d[:]], outs=[h1full_d[:]])
                    continue
                layer(1, xfull_d, F_IN, wl1_t, wr1_t, b1_t, xt_t, h1T_t)
                if mode == "l1":
                    nc.sync.dma_start(
                        out_d.bitcast(BF16)[:, :NL], h1T_t[:])
                    continue
                nc.gpsimd.collective_compute(
                    "AllGather", mybir.AluOpType.bypass,
                    replica_groups=[list(range(P))],
                    ins=[h1pad_d[:]], outs=[h1full_d[:]])
                if mode == "l1+ag":
                    nc.sync.dma_start(
                        out_d.bitcast(BF16)[:, :NL], h1T_t[:])
                    continue
                layer(2, h1full_d, F_OUT, wl2_t, wr2_t, b2_t, h1T_t, None)
            if mode in ("g1", "agonly", "l1p", "g1s", "l1t", "l1w"):
                nc.sync.dma_start(
                    out_d.bitcast(BF16)[:, :NL], xt_t[:F_OUT, :])
            elif mode in ("c1", "c1ns", "l1r", "gser"):
                nc.sync.dma_start(
                    out_d.bitcast(BF16)[:, :NL], h1T_t[:])

    nc.finalize()
    return nc


ICH = 64   # pairs per bulk-indicator build


def _build3(meta, rep=1, mode="full"):
    """v3: bulk indicator builds (tensor_tensor is_equal over broadcast APs)
    + node-major accumulation acc[dst,F] = ind01^T @ stage with 1/deg as a
    per-partition Act scale at PSUM drain. ~40 DVE instructions per layer
    instead of ~2200 (per-pair tensor_scalar interleaved with SWDGE gathers
    measured ~450ns each of sem/dispatch poison; bulk builds sidestep it).
    """
    calls = meta["calls"]
    pairs = meta["pairs"]
    MAXSUPC = meta["MAXSUPC"]
    NPAIR = meta["NPAIR"]
    IDX_COLS = meta["IDX_COLS"]

    nc = bacc.Bacc("TRN2", target_bir_lowering=False, debug=False,
                   num_devices=P, num_swdge_queues=4)
    dt = mybir.dt
    xfull_d = nc.dram_tensor("xfull", [NG, F_IN], BF16, kind="ExternalInput")
    xt_d = nc.dram_tensor("xt", [F_IN, NL], BF16, kind="ExternalInput")
    idx_d = nc.dram_tensor("idx", [128, IDX_COLS], dt.int16,
                           kind="ExternalInput")
    seg_d = nc.dram_tensor("seg", [128, NPAIR], BF16, kind="ExternalInput")
    rvn_d = nc.dram_tensor("rvn", [128, NB], dt.float32,
                           kind="ExternalInput")
    iota_d = nc.dram_tensor("iota", [128, BLK], BF16, kind="ExternalInput")
    wl1_d = nc.dram_tensor("W_l1", [F_IN, F_OUT], BF16, kind="ExternalInput")
    wr1_d = nc.dram_tensor("W_r1", [F_IN, F_OUT], BF16, kind="ExternalInput")
    b1_d = nc.dram_tensor("b1", [1, F_OUT], BF16, kind="ExternalInput")
    wl2_d = nc.dram_tensor("W_l2", [F_OUT, F_OUT], BF16,
                           kind="ExternalInput")
    wr2_d = nc.dram_tensor("W_r2", [F_OUT, F_OUT], BF16,
                           kind="ExternalInput")
    b2_d = nc.dram_tensor("b2", [1, F_OUT], BF16, kind="ExternalInput")
    out_d = nc.dram_tensor("out", [NL, F_OUT], dt.float32,
                           kind="ExternalOutput")

    h1pad_d = nc.dram_tensor("h1pad", [NL, F_IN], BF16)
    h1full_d = nc.dram_tensor("h1full", [NG, F_IN], BF16, addr_space="Shared")

    # super s covers pair range [srange[s], srange[s+1])
    srange = [pairs[s * SBK][0][1] for s in range(NSUP)] + [NPAIR]

    with tile.TileContext(nc) as tc:
        with (
            tc.tile_pool(name="const", bufs=1) as constp,
            tc.tile_pool(name="indall", bufs=3) as indall,
            tc.tile_pool(name="op", bufs=4) as op,
            tc.tile_pool(name="ps_acc", bufs=2, space="PSUM") as ps_acc,
            tc.tile_pool(name="ps_t", bufs=2, space="PSUM") as ps_t,
            tc.tile_pool(name="ps_o", bufs=2, space="PSUM") as ps_o,
        ):
            iota_t = constp.tile([128, BLK], BF16)
            nc.sync.dma_start(iota_t[:], iota_d[:])
            seg_t = constp.tile([128, NPAIR], BF16)
            nc.sync.dma_start(seg_t[:], seg_d[:])
            rvn_t = constp.tile([128, NB], dt.float32)
            nc.sync.dma_start(rvn_t[:], rvn_d[:])
            idx_t = constp.tile([128, IDX_COLS], dt.int16)
            nc.sync.dma_start(idx_t[:], idx_d[:])
            xt_t = constp.tile([F_IN, NL], BF16)
            nc.sync.dma_start(xt_t[:], xt_d[:])
            wl1_t = constp.tile([F_IN, F_OUT], BF16)
            nc.sync.dma_start(wl1_t[:], wl1_d[:])
            wr1_t = constp.tile([F_IN, F_OUT], BF16)
            nc.sync.dma_start(wr1_t[:], wr1_d[:])
            wl2_t = constp.tile([F_OUT, F_OUT], BF16)
            nc.sync.dma_start(wl2_t[:], wl2_d[:])
            wr2_t = constp.tile([F_OUT, F_OUT], BF16)
            nc.sync.dma_start(wr2_t[:], wr2_d[:])
            b1_t = constp.tile([1, F_OUT], BF16)
            nc.sync.dma_start(b1_t[:], b1_d[:])
            b2_t = constp.tile([1, F_OUT], BF16)
            nc.sync.dma_start(b2_t[:], b2_d[:])
            ones_t = constp.tile([1, BLK], BF16)
            nc.vector.memset(ones_t[:], 1.0)
            from concourse.masks import make_identity
            id128_t = constp.tile([128, 128], BF16)
            make_identity(nc, id128_t[:])
            h1T_t = constp.tile([F_OUT, NL], BF16)

            stage_a = constp.tile([128, MAXSUPC * F_IN], BF16)
            stage_b = constp.tile([128, MAXSUPC * F_IN], BF16)
            stage = [stage_a, stage_b]
            nc.gpsimd.memset(stage[0][:], 0.0)
            nc.gpsimd.memset(stage[1][:], 0.0)

            qn = [0]
            ident = mybir.ActivationFunctionType

            def gathers(s, table):
                buf = stage[s % 2]
                for (ss, c, loc0, cols, ioff) in calls:
                    if ss != s:
                        continue
                    nc.gpsimd.dma_gather(
                        out_ap=buf[:, loc0 * F_IN:(loc0 + cols) * F_IN]
                            .rearrange("p (c f) -> p c f", f=F_IN),
                        in_ap=table[c * CHUNK:min((c + 1) * CHUNK, NG), :],
                        idxs_ap=idx_t[:, ioff:ioff + cols * 8],
                        num_idxs=cols * 128, num_idxs_reg=cols * 128,
                        elem_size=F_IN, single_packet=True,
                        queue_num=qn[0] % 4)
                    qn[0] += 1

            def ind_builds(s):
                # bulk 0/1 indicators for all pairs of super s
                plo, phi = srange[s], srange[s + 1]
                cmap = {}
                for c0 in range(plo, phi, ICH):
                    np_ = min(ICH, phi - c0)
                    ich = indall.tile([128, ICH * 128], BF16, tag="ich")
                    nc.vector.tensor_tensor(
                        out=ich[:, :np_ * 128]
                            .rearrange("p (n j) -> p n j", j=128),
                        in0=seg_t[:, c0:c0 + np_].unsqueeze(2)
                            .broadcast_to([128, np_, 128]),
                        in1=iota_t[:].unsqueeze(1)
                            .broadcast_to([128, np_, BLK]),
                        op=mybir.AluOpType.is_equal)
                    for p in range(c0, c0 + np_):
                        cmap[p] = (ich, p - c0)
                return cmap

            def layer(li, table, FW, wl_t, wr_t, bias_t, selfT,
                      ag_hook=None):
                for s in range(NSUP):
                    if s == 0:
                        gathers(s, table)
                    if s + 1 < NSUP:
                        gathers(s + 1, table)
                    cmap = ind_builds(s)
                    buf = stage[s % 2]
                    for b in range(s * SBK, (s + 1) * SBK):
                        pl = pairs[b]
                        acc = ps_acc.tile([BLK, FW], dt.float32, tag="acc")
                        for j, (loc, pcol) in enumerate(pl):
                            it, off = cmap[pcol]
                            nc.tensor.matmul(
                                acc[:],
                                lhsT=it[:, off * 128:(off + 1) * 128],
                                rhs=buf[:, loc * F_IN:loc * F_IN + FW],
                                start=(j == 0), stop=(j == len(pl) - 1))
                        meanN = op.tile([BLK, FW], BF16, tag="meanN")
                        nc.scalar.activation(out=meanN[:], in_=acc[:],
                                             func=ident.Copy,
                                             scale=rvn_t[:, b:b + 1])
                        trm = ps_t.tile([FW, BLK], BF16, tag="trm")
                        nc.tensor.transpose(out=trm[:], in_=meanN[:],
                                            identity=id128_t[:])
                        meanT = op.tile([FW, BLK], BF16, tag="meanT")
                        nc.scalar.activation(out=meanT[:], in_=trm[:],
                                             func=ident.Copy)
                        o2 = ps_o.tile([BLK, F_OUT], dt.float32, tag="o2")
                        nc.tensor.matmul(o2[:], lhsT=meanT[:], rhs=wl_t[:],
                                         start=True, stop=False)
                        nc.tensor.matmul(o2[:],
                                         lhsT=selfT[:, b * BLK:(b + 1) * BLK],
                                         rhs=wr_t[:],
                                         start=False, stop=False)
                        nc.tensor.matmul(o2[:], lhsT=ones_t[:1, :],
                                         rhs=bias_t[:1, :],
                                         start=False, stop=True)
                        if li == 1:
                            h1n = op.tile([BLK, F_OUT], BF16, tag="h1n")
                            nc.scalar.activation(out=h1n[:], in_=o2[:],
                                                 func=ident.Relu)
                            nc.sync.dma_start(
                                h1pad_d[b * BLK:(b + 1) * BLK, :F_OUT],
                                h1n[:])
                            trh = ps_t.tile([F_OUT, BLK], BF16, tag="trh")
                            nc.tensor.transpose(out=trh[:], in_=h1n[:],
                                                identity=id128_t[:])
                            nc.scalar.activation(
                                out=h1T_t[:, b * BLK:(b + 1) * BLK],
                                in_=trh[:], func=ident.Copy)
                        else:
                            ob = op.tile([BLK, F_OUT], dt.float32, tag="ob")
                            nc.scalar.activation(out=ob[:], in_=o2[:],
                                                 func=ident.Copy)
                            nc.sync.dma_start(
                                out_d[b * BLK:(b + 1) * BLK, :], ob[:])
                    if ag_hook is not None:
                        ag_hook(s)

            def ag_group(g):
                a = g * GRP_ROWS
                nc.gpsimd.collective_compute(
                    "AllGather", mybir.AluOpType.bypass,
                    replica_groups=[list(range(P))],
                    ins=[h1pad_d[a:a + GSZ[g], :]],
                    outs=[h1full_d[GBASE[g]:GBASE[g] + P * GSZ[g], :]])

            for _r in range(rep):
                bounds = list(range(AGS, NSUP, AGS)) + [NSUP]

                def hook(s):
                    if s + 1 in bounds:
                        ag_group(bounds.index(s + 1))

                layer(1, xfull_d, F_IN, wl1_t, wr1_t, b1_t, xt_t,
                      ag_hook=hook)
                layer(2, h1full_d, F_OUT, wl2_t, wr2_t, b2_t, h1T_t)

    nc.finalize()
    return nc


def _make_inputs3(x, W_l1, W_r1, b1, W_l2, W_r2, b2, meta):
    x = np.asarray(x, dtype=np.float32)
    x_full = np.zeros((NG, F_IN), dtype=np.float32)
    slots = np.arange(NREAL)
    for k in range(P):
        x_full[_gmap(k, slots)] = x[k * NREAL:(k + 1) * NREAL]
    x_full_bf = x_full.astype(NPBF16)
    xpad = np.zeros((P, NL, F_IN), dtype=np.float32)
    for k in range(P):
        xpad[k, :NREAL] = x[k * NREAL:(k + 1) * NREAL]
    xpad_bf = xpad.astype(NPBF16)
    iota = np.broadcast_to(np.arange(BLK, dtype=np.float32),
                           (128, BLK)).astype(NPBF16).copy()
    in_maps = []
    for k in range(P):
        in_maps.append({
            "xfull": x_full_bf,
            "xt": np.ascontiguousarray(xpad_bf[k].T),
            "idx": meta["idx"][k],
            "seg": meta["segb"][k],
            "rvn": meta["rvn"][k],
            "iota": iota,
            "W_l1": np.asarray(W_l1, np.float32).astype(NPBF16),
            "W_r1": np.asarray(W_r1, np.float32).astype(NPBF16),
            "b1": np.asarray(b1, np.float32).reshape(1, F_OUT).astype(NPBF16),
            "W_l2": np.asarray(W_l2, np.float32).astype(NPBF16),
            "W_r2": np.asarray(W_r2, np.float32).astype(NPBF16),
            "b2": np.asarray(b2, np.float32).reshape(1, F_OUT).astype(NPBF16),
        })
    return in_maps


def _make_inputs(x, W_l1, W_r1, b1, W_l2, W_r2, b2, meta):
    x = np.asarray(x, dtype=np.float32)
    x_full = np.zeros((NG, F_IN), dtype=np.float32)
    for k in range(P):
        x_full[k * NL:k * NL + NREAL] = x[k * NREAL:(k + 1) * NREAL]
    x_full_bf = x_full.astype(NPBF16)
    iota = np.broadcast_to(np.arange(BLK, dtype=np.float32),
                           (128, BLK)).astype(NPBF16).copy()
    in_maps = []
    for k in range(P):
        in_maps.append({
            "xfull": x_full_bf,
            "xt": np.ascontiguousarray(x_full_bf[k * NL:(k + 1) * NL].T),
            "idx": meta["idx"][k],
            "seg": meta["seg"][k],
            "rv": meta["rv"][k],
            "iota": iota,
            "W_l1": np.asarray(W_l1, np.float32).astype(NPBF16),
            "W_r1": np.asarray(W_r1, np.float32).astype(NPBF16),
            "b1": np.asarray(b1, np.float32).reshape(1, F_OUT).astype(NPBF16),
            "W_l2": np.asarray(W_l2, np.float32).astype(NPBF16),
            "W_r2": np.asarray(W_r2, np.float32).astype(NPBF16),
            "b2": np.asarray(b2, np.float32).reshape(1, F_OUT).astype(NPBF16),
        })
    return in_maps


def kernel(x, edge_index, W_l1, W_r1, b1, W_l2, W_r2, b2):
    meta = _preprocess(np.asarray(edge_index))
    in_maps = _make_inputs3(x, W_l1, W_r1, b1, W_l2, W_r2, b2, meta)
    nc = _build3(meta)
    res = run_bass_kernel_spmd(nc, in_maps, core_ids=list(range(P)))
    out = np.concatenate(
        [res.results[k]["out"][:NREAL] for k in range(P)], axis=0)
    return out.astype(np.float32)


if __name__ == "__main__":
    rng = np.random.default_rng(0)
    x = rng.normal(size=(N_NODES, F_IN)).astype(np.float32)
    ei = rng.integers(0, N_NODES, size=(2, N_EDGES)).astype(np.int64)
    wl1 = rng.normal(size=(F_IN, F_OUT)).astype(np.float32) / np.sqrt(F_IN)
    wr1 = rng.normal(size=(F_IN, F_OUT)).astype(np.float32) / np.sqrt(F_IN)
    wl2 = rng.normal(size=(F_OUT, F_OUT)).astype(np.float32) / np.sqrt(F_OUT)
    wr2 = rng.normal(size=(F_OUT, F_OUT)).astype(np.float32) / np.sqrt(F_OUT)
    b1 = np.zeros(F_OUT, np.float32)
    b2 = np.zeros(F_OUT, np.float32)
    out = kernel(x, ei, wl1, wr1, b1, wl2, wr2, b2)
    print("out", out.shape, out.dtype, float(np.abs(out).mean()))



# revision 24
# speedup vs baseline: 1.2663x; 1.0774x over previous
"""2-layer GraphSAGE (mean aggregation) on 8 Trainium2 NeuronCores — v2.

Strategy (dst-sharded graph parallel), changes vs v1:
- bf16 data path: x table, staged gathers, indicators, weight matmuls (PE
  1cyc/row vs fp32 4), PSUM accumulate fp32.
- Transposed accumulation: acc[F, dst] = stage[pos, F].T @ ind[pos, dst];
  1/deg folded into the indicator value (tensor_scalar is_equal * rv), so
  PSUM holds mean^T directly -> no per-block scale/transpose chain.
- Superblock gather calls: ~4096 indices/call (vs 512) -> ~8x fewer SWDGE
  fixed overheads on Pool. Trailing pad indices are -1 (trimmed by ucode,
  no descriptors) instead of gathering row 0.
- Both layers share one idx/seg/rv tensor set (same edge structure, both
  tables are 256B-row bf16 [NG, 128]).
- Layer outputs are produced transposed [64, NL]; host untransposes.
"""
import sys
sys.path.insert(0, "/opt/trn_rl_repo")
import numpy as np

import concourse.bass as bass
import concourse.bacc as bacc
import concourse.mybir as mybir
import concourse.tile as tile
from concourse.bass_utils import run_bass_kernel_spmd

N_NODES = 100000
N_EDGES = 1600000
F_IN = 128
F_OUT = 64
P = 8
NREAL = 12500
NL = 12544            # 98 * 128
BLK = 128
NB = NL // BLK        # 98
SBK = 7               # blocks per superblock
NSUP = NB // SBK      # 14
SUPN = SBK * BLK      # 896 dsts per super
CHUNK = 32768
NCHUNK = (P * NL + CHUNK - 1) // CHUNK   # 4
NG = P * NL           # 100352
GCOLS = 8             # max 128-idx cols per gather call (1024 idxs).
                      # Empirical HW limits: 2048-idx calls deadlock the
                      # SWDGE ring (129 descs/engine > 128 in-flight cap);
                      # 1536/1920 also fail (Q7 idx scratch); 1024 is stable.
SENT = 999.0
AGS = 14                             # supers per AllGather group
GRP_ROWS = AGS * SUPN                # 3584
NGRP = (NSUP + AGS - 1) // AGS       # 4
GSZ = [min(NSUP, (g + 1) * AGS) * SUPN - g * GRP_ROWS for g in range(NGRP)]
GBASE = [0] * NGRP                   # global row base of each group
for _g in range(1, NGRP):
    GBASE[_g] = GBASE[_g - 1] + P * GSZ[_g - 1]


def _gmap(core, slot):
    # group-major global table row for (core, local slot)
    g = np.minimum(slot // GRP_ROWS, NGRP - 1)
    gsz = np.asarray(GSZ)[g]
    gbase = np.asarray(GBASE)[g]
    return gbase + core * gsz + (slot - g * GRP_ROWS)

BF16 = mybir.dt.bfloat16
NPBF16 = mybir.dt.np(BF16)


def _wrap16(flat_idx):
    w = flat_idx.reshape(-1, 16).T.copy()
    return np.tile(w, (8, 1))


def _preprocess(edge_index):
    src = np.asarray(edge_index[0], dtype=np.int64)
    dst = np.asarray(edge_index[1], dtype=np.int64)
    dcore = dst // NREAL
    dslot = dst - dcore * NREAL
    score = src // NREAL
    g_src = _gmap(score, src - score * NREAL)

    cores = []           # per core dict: ds, g (sorted), seg boundaries
    for k in range(P):
        sel = dcore == k
        ds = dslot[sel]
        g = g_src[sel]
        ch = g // CHUNK
        sup = ds // SUPN
        order = np.lexsort((g, ds, ch, sup))
        ds, g, ch, sup = ds[order], g[order], ch[order], sup[order]
        code = sup * NCHUNK + ch
        bounds = np.searchsorted(code, np.arange(NSUP * NCHUNK + 1))
        cnt = np.bincount(ds, minlength=NL).astype(np.float64)
        rv = (1.0 / np.maximum(cnt, 1.0)).astype(np.float32)
        cores.append(dict(ds=ds, g=g, bounds=bounds, rv=rv))

    # uniform cols per (sup, chunk)
    ncols = np.zeros((NSUP, NCHUNK), dtype=np.int64)
    for k in range(P):
        b = cores[k]["bounds"]
        n = (b[1:] - b[:-1]).reshape(NSUP, NCHUNK)
        ncols = np.maximum(ncols, (n + 127) // 128)
    seg_col0 = np.zeros((NSUP, NCHUNK), dtype=np.int64)   # global col base
    sup_col0 = np.zeros(NSUP, dtype=np.int64)             # col base within super
    tot = 0
    for s in range(NSUP):
        loc = 0
        for c in range(NCHUNK):
            seg_col0[s, c] = loc          # local to super
            loc += int(ncols[s, c])
        sup_col0[s] = tot
        tot += loc
    sup_cols = [int(ncols[s].sum()) for s in range(NSUP)]
    MAXSUPC = max(sup_cols)

    # calls: (s, c, loc_col0, cols, idx_off) — uniform
    calls = []
    idx_off = 0
    for s in range(NSUP):
        for c in range(NCHUNK):
            nc_ = int(ncols[s, c])
            done = 0
            while done < nc_:
                piece = min(GCOLS, nc_ - done)
                calls.append((s, c, int(seg_col0[s, c]) + done, piece, idx_off))
                idx_off += piece * 8
                done += piece
    IDX_COLS = idx_off

    # block windows (uniform): for each block b, chunk c -> [wlo, whi) local cols
    wins = np.zeros((NB, NCHUNK, 2), dtype=np.int64)
    wins[:, :, 0] = 1 << 60
    for k in range(P):
        ds, bounds = cores[k]["ds"], cores[k]["bounds"]
        for s in range(NSUP):
            for c in range(NCHUNK):
                s0, s1 = bounds[s * NCHUNK + c], bounds[s * NCHUNK + c + 1]
                blkseg = ds[s0:s1] // BLK
                for b in range(s * SBK, (s + 1) * SBK):
                    lo = int(np.searchsorted(blkseg, b))
                    hi = int(np.searchsorted(blkseg, b + 1))
                    if hi > lo:
                        wins[b, c, 0] = min(wins[b, c, 0], lo // 128)
                        wins[b, c, 1] = max(wins[b, c, 1], (hi + 127) // 128)

    # pairs: per block, list of (loc_col, pair_idx)
    pairs = [[] for _ in range(NB)]
    npair = 0
    for b in range(NB):
        s = b // SBK
        for c in range(NCHUNK):
            wlo, whi = wins[b, c]
            if whi <= wlo:
                continue
            for t in range(int(wlo), int(whi)):
                pairs[b].append((int(seg_col0[s, c]) + t, npair))
                npair += 1
    NPAIR = npair

    # per-core tensors
    idx_cores, seg_cores, rv_cores = [], [], []
    for k in range(P):
        ds, g, bounds, rv = (cores[k][x] for x in ("ds", "g", "bounds", "rv"))
        seg_np = np.full((NPAIR, 128), SENT, dtype=np.float32)
        rv_np = np.zeros((NPAIR, 128), dtype=np.float32)
        # pad positions gather row 0 (cheap, finite); sentinel seg zeroes
        # their contribution. Negative (skipped) indices desync the SWDGE
        # ring bookkeeping (decode reserves untrimmed, gen trims) -> hang.
        idx_flat = np.zeros(tot * 128, dtype=np.int64)
        for s in range(NSUP):
            for c in range(NCHUNK):
                s0, s1 = bounds[s * NCHUNK + c], bounds[s * NCHUNK + c + 1]
                n = s1 - s0
                base = (sup_col0[s] + seg_col0[s, c]) * 128
                idx_flat[base:base + n] = g[s0:s1] - c * CHUNK
        idx_cores.append(idx_flat)
        seg_cores.append(seg_np)
        rv_cores.append(rv_np)

    # fill seg/rv per pair (redo with pair indices known)
    pair_list = []   # (b, c, t_local)
    for b in range(NB):
        s = b // SBK
        for c in range(NCHUNK):
            wlo, whi = wins[b, c]
            for t in range(int(wlo), int(whi)):
                pair_list.append((b, c, t))
    assert len(pair_list) == NPAIR
    for k in range(P):
        ds, bounds, rv = (cores[k][x] for x in ("ds", "bounds", "rv"))
        seg_np = seg_cores[k]
        rv_np = rv_cores[k]
        for p, (b, c, t) in enumerate(pair_list):
            s = b // SBK
            s0, s1 = bounds[s * NCHUNK + c], bounds[s * NCHUNK + c + 1]
            n = int(s1 - s0)
            p0 = t * 128
            p1 = min(p0 + 128, n)
            if p1 <= p0:
                continue
            dseg = ds[s0 + p0:s0 + p1]
            m = (dseg // BLK) == b
            col_s = seg_np[p]
            col_r = rv_np[p]
            idxs = np.nonzero(m)[0]
            col_s[idxs] = (dseg[m] - b * BLK).astype(np.float32)
            col_r[idxs] = rv[dseg[m]]

    # wrap idx into per-call int16 layout
    idx_out = []
    for k in range(P):
        flat = idx_cores[k]
        parts = []
        for (s, c, loc0, cols, _off) in calls:
            base = (sup_col0[s] + loc0) * 128
            parts.append(_wrap16(flat[base:base + cols * 128].astype(np.int16)))
        idx_out.append(np.concatenate(parts, axis=1))
        assert idx_out[-1].shape == (128, IDX_COLS)

    seg_out = [np.ascontiguousarray(s.T) for s in seg_cores]   # [128, NPAIR] fp32
    rv_out = [np.ascontiguousarray(r.T) for r in rv_cores]
    segb_out = [s.astype(NPBF16) for s in seg_out]             # [128, NPAIR] bf16
    rvn_out = []                                               # [128, NB] fp32
    for k in range(P):
        rvn_out.append(np.ascontiguousarray(
            cores[k]["rv"].reshape(NB, BLK).T))

    return dict(calls=calls, pairs=pairs, sup_col0=sup_col0, sup_cols=sup_cols,
                MAXSUPC=MAXSUPC, NPAIR=NPAIR, IDX_COLS=IDX_COLS,
                idx=idx_out, seg=seg_out, rv=rv_out, segb=segb_out,
                rvn=rvn_out)


def _build(meta, mode="full", rep=1):
    calls = meta["calls"]
    pairs = meta["pairs"]
    sup_col0 = meta["sup_col0"]
    MAXSUPC = meta["MAXSUPC"]
    NPAIR = meta["NPAIR"]
    IDX_COLS = meta["IDX_COLS"]

    nc = bacc.Bacc("TRN2", target_bir_lowering=False, debug=False,
                   num_devices=P, num_swdge_queues=4)
    dt = mybir.dt
    xfull_d = nc.dram_tensor("xfull", [NG, F_IN], BF16, kind="ExternalInput")
    xt_d = nc.dram_tensor("xt", [F_IN, NL], BF16, kind="ExternalInput")
    idx_d = nc.dram_tensor("idx", [128, IDX_COLS], dt.int16, kind="ExternalInput")
    seg_d = nc.dram_tensor("seg", [128, NPAIR], dt.float32, kind="ExternalInput")
    rv_d = nc.dram_tensor("rv", [128, NPAIR], dt.float32, kind="ExternalInput")
    iota_d = nc.dram_tensor("iota", [128, BLK], BF16, kind="ExternalInput")
    wl1_d = nc.dram_tensor("W_l1", [F_IN, F_OUT], BF16, kind="ExternalInput")
    wr1_d = nc.dram_tensor("W_r1", [F_IN, F_OUT], BF16, kind="ExternalInput")
    b1_d = nc.dram_tensor("b1", [1, F_OUT], BF16, kind="ExternalInput")
    wl2_d = nc.dram_tensor("W_l2", [F_OUT, F_OUT], BF16, kind="ExternalInput")
    wr2_d = nc.dram_tensor("W_r2", [F_OUT, F_OUT], BF16, kind="ExternalInput")
    b2_d = nc.dram_tensor("b2", [1, F_OUT], BF16, kind="ExternalInput")
    out_d = nc.dram_tensor("out", [F_OUT, NL], dt.float32, kind="ExternalOutput")

    h1pad_d = nc.dram_tensor("h1pad", [NL, F_IN], BF16)
    h1full_d = nc.dram_tensor("h1full", [NG, F_IN], BF16, addr_space="Shared")

    with tile.TileContext(nc) as tc:
        acc_bufs = 4 if mode in ("l1r",) else 2
        with (
            tc.tile_pool(name="const", bufs=1) as constp,
            tc.tile_pool(name="indp", bufs=8) as indp,
            tc.tile_pool(name="op", bufs=4) as op,
            tc.tile_pool(name="ps_acc", bufs=acc_bufs, space="PSUM") as ps_acc,
            tc.tile_pool(name="ps_t", bufs=2, space="PSUM") as ps_t,
            tc.tile_pool(name="ps_o", bufs=2, space="PSUM") as ps_o,
        ):
            iota_t = constp.tile([128, BLK], BF16)
            nc.sync.dma_start(iota_t[:], iota_d[:])
            seg_t = constp.tile([128, NPAIR], dt.float32)
            nc.sync.dma_start(seg_t[:], seg_d[:])
            rv_t = constp.tile([128, NPAIR], dt.float32)
            nc.sync.dma_start(rv_t[:], rv_d[:])
            idx_t = constp.tile([128, IDX_COLS], dt.int16)
            nc.sync.dma_start(idx_t[:], idx_d[:])
            xt_t = constp.tile([F_IN, NL], BF16)
            nc.sync.dma_start(xt_t[:], xt_d[:])
            wl1_t = constp.tile([F_IN, F_OUT], BF16)
            nc.sync.dma_start(wl1_t[:], wl1_d[:])
            wr1_t = constp.tile([F_IN, F_OUT], BF16)
            nc.sync.dma_start(wr1_t[:], wr1_d[:])
            wl2_t = constp.tile([F_OUT, F_OUT], BF16)
            nc.sync.dma_start(wl2_t[:], wl2_d[:])
            wr2_t = constp.tile([F_OUT, F_OUT], BF16)
            nc.sync.dma_start(wr2_t[:], wr2_d[:])
            b1_t = constp.tile([1, F_OUT], BF16)
            nc.sync.dma_start(b1_t[:], b1_d[:])
            b2_t = constp.tile([1, F_OUT], BF16)
            nc.sync.dma_start(b2_t[:], b2_d[:])
            ones_t = constp.tile([1, BLK], BF16)
            nc.vector.memset(ones_t[:], 1.0)
            from concourse.masks import make_identity
            id_t = constp.tile([F_OUT, F_OUT], BF16)
            make_identity(nc, id_t[:])
            h1T_t = constp.tile([F_OUT, NL], BF16)

            stage_a = constp.tile([128, MAXSUPC * F_IN], BF16)
            stage_b = constp.tile([128, MAXSUPC * F_IN], BF16)
            stage = [stage_a, stage_b]
            nc.gpsimd.memset(stage[0][:], 0.0)
            nc.gpsimd.memset(stage[1][:], 0.0)
            if mode in ("gser", "g1s"):
                stage_c = constp.tile([128, MAXSUPC * F_IN], BF16)
                dummy = [stage_c, stage_c]

            qn = [0]
            ident = mybir.ActivationFunctionType

            def gathers(s, table, into=None):
                buf = (into or stage)[s % 2]
                for (ss, c, loc0, cols, ioff) in calls:
                    if ss != s:
                        continue
                    nc.gpsimd.dma_gather(
                        out_ap=buf[:, loc0 * F_IN:(loc0 + cols) * F_IN]
                            .rearrange("p (c f) -> p c f", f=F_IN),
                        in_ap=table[c * CHUNK:min((c + 1) * CHUNK, NG), :],
                        idxs_ap=idx_t[:, ioff:ioff + cols * 8],
                        num_idxs=cols * 128, num_idxs_reg=cols * 128,
                        elem_size=F_IN, single_packet=True,
                        queue_num=qn[0] % 4)
                    qn[0] += 1

            def layer(li, table, FW, wl_t, wr_t, bias_t, selfT, out_sb):
                for s in range(NSUP):
                    if s == 0:
                        gathers(s, table)
                    if s + 1 < NSUP:
                        gathers(s + 1, table)
                    buf = stage[s % 2]
                    for b in range(s * SBK, (s + 1) * SBK):
                        pl = pairs[b]
                        acc = ps_acc.tile([FW, BLK], dt.float32, tag="acc")
                        for j, (loc, pcol) in enumerate(pl):
                            ind = indp.tile([128, BLK], BF16, tag="ind")
                            nc.vector.tensor_scalar(
                                out=ind[:], in0=iota_t[:],
                                scalar1=seg_t[:, pcol:pcol + 1],
                                scalar2=rv_t[:, pcol:pcol + 1],
                                op0=mybir.AluOpType.is_equal,
                                op1=mybir.AluOpType.mult)
                            nc.tensor.matmul(
                                acc[:],
                                lhsT=buf[:, loc * F_IN:loc * F_IN + FW],
                                rhs=ind[:],
                                start=(j == 0), stop=(j == len(pl) - 1))
                        meanT = op.tile([FW, BLK], BF16, tag="meanT")
                        nc.scalar.activation(out=meanT[:], in_=acc[:],
                                             func=ident.Copy)
                        o_ps = ps_o.tile([F_OUT, BLK], dt.float32, tag="ops")
                        nc.tensor.matmul(o_ps[:], lhsT=wl_t[:], rhs=meanT[:],
                                         start=True, stop=False)
                        nc.tensor.matmul(o_ps[:], lhsT=wr_t[:],
                                         rhs=selfT[:, b * BLK:(b + 1) * BLK],
                                         start=False, stop=False)
                        nc.tensor.matmul(o_ps[:], lhsT=bias_t[:1, :],
                                         rhs=ones_t[:1, :],
                                         start=False, stop=True)
                        if li == 1:
                            # h1T block (bf16, relu) kept in SBUF for L2 self
                            nc.scalar.activation(
                                out=out_sb[:, b * BLK:(b + 1) * BLK],
                                in_=o_ps[:], func=ident.Relu)
                            # node-major bf16 copy for the gather table
                            tr = ps_t.tile([BLK, F_OUT], BF16, tag="tr")
                            nc.tensor.transpose(
                                out=tr[:],
                                in_=out_sb[:, b * BLK:(b + 1) * BLK],
                                identity=id_t[:])
                            h1n = op.tile([BLK, F_OUT], BF16, tag="h1n")
                            nc.scalar.activation(out=h1n[:], in_=tr[:],
                                                 func=ident.Copy)
                            nc.sync.dma_start(
                                h1pad_d[b * BLK:(b + 1) * BLK, :F_OUT], h1n[:])
                        else:
                            ob = op.tile([F_OUT, BLK], dt.float32, tag="ob")
                            nc.scalar.activation(out=ob[:], in_=o_ps[:],
                                                 func=ident.Copy)
                            nc.sync.dma_start(
                                out_d[:, b * BLK:(b + 1) * BLK], ob[:])

            def gathers_only():
                for s in range(NSUP):
                    gathers(s, xfull_d)

            def compute_only():
                # L1 pipeline minus the gather calls (stage holds garbage)
                for s in range(NSUP):
                    for b in range(s * SBK, (s + 1) * SBK):
                        pl = pairs[b]
                        acc = ps_acc.tile([F_IN, BLK], dt.float32, tag="acc")
                        for j, (loc, pcol) in enumerate(pl):
                            ind = indp.tile([128, BLK], BF16, tag="ind")
                            nc.vector.tensor_scalar(
                                out=ind[:], in0=iota_t[:],
                                scalar1=seg_t[:, pcol:pcol + 1],
                                scalar2=rv_t[:, pcol:pcol + 1],
                                op0=mybir.AluOpType.is_equal,
                                op1=mybir.AluOpType.mult)
                            nc.tensor.matmul(
                                acc[:],
                                lhsT=stage[s % 2][:, loc * F_IN:
                                                  loc * F_IN + F_IN],
                                rhs=ind[:],
                                start=(j == 0), stop=(j == len(pl) - 1))
                        meanT = op.tile([F_IN, BLK], BF16, tag="meanT")
                        nc.scalar.activation(out=meanT[:], in_=acc[:],
                                             func=ident.Copy)
                        o_ps = ps_o.tile([F_OUT, BLK], dt.float32, tag="ops")
                        nc.tensor.matmul(o_ps[:], lhsT=wl1_t[:], rhs=meanT[:],
                                         start=True, stop=False)
                        nc.tensor.matmul(o_ps[:], lhsT=wr1_t[:],
                                         rhs=xt_t[:, b * BLK:(b + 1) * BLK],
                                         start=False, stop=False)
                        nc.tensor.matmul(o_ps[:], lhsT=b1_t[:1, :],
                                         rhs=ones_t[:1, :],
                                         start=False, stop=True)
                        nc.scalar.activation(
                            out=h1T_t[:, b * BLK:(b + 1) * BLK],
                            in_=o_ps[:], func=ident.Relu)
                        tr = ps_t.tile([BLK, F_OUT], BF16, tag="tr")
                        nc.tensor.transpose(
                            out=tr[:],
                            in_=h1T_t[:, b * BLK:(b + 1) * BLK],
                            identity=id_t[:])
                        h1n = op.tile([BLK, F_OUT], BF16, tag="h1n")
                        nc.scalar.activation(out=h1n[:], in_=tr[:],
                                             func=ident.Copy)
                        nc.sync.dma_start(
                            h1pad_d[b * BLK:(b + 1) * BLK, :F_OUT], h1n[:])

            def compute_nostore():
                for s in range(NSUP):
                    for b in range(s * SBK, (s + 1) * SBK):
                        pl = pairs[b]
                        acc = ps_acc.tile([F_IN, BLK], dt.float32, tag="acc")
                        for j, (loc, pcol) in enumerate(pl):
                            ind = indp.tile([128, BLK], BF16, tag="ind")
                            nc.vector.tensor_scalar(
                                out=ind[:], in0=iota_t[:],
                                scalar1=seg_t[:, pcol:pcol + 1],
                                scalar2=rv_t[:, pcol:pcol + 1],
                                op0=mybir.AluOpType.is_equal,
                                op1=mybir.AluOpType.mult)
                            nc.tensor.matmul(
                                acc[:],
                                lhsT=stage[s % 2][:, loc * F_IN:
                                                  loc * F_IN + F_IN],
                                rhs=ind[:],
                                start=(j == 0), stop=(j == len(pl) - 1))
                        meanT = op.tile([F_IN, BLK], BF16, tag="meanT")
                        nc.scalar.activation(out=meanT[:], in_=acc[:],
                                             func=ident.Copy)
                        o_ps = ps_o.tile([F_OUT, BLK], dt.float32, tag="ops")
                        nc.tensor.matmul(o_ps[:], lhsT=wl1_t[:], rhs=meanT[:],
                                         start=True, stop=False)
                        nc.tensor.matmul(o_ps[:], lhsT=wr1_t[:],
                                         rhs=xt_t[:, b * BLK:(b + 1) * BLK],
                                         start=False, stop=False)
                        nc.tensor.matmul(o_ps[:], lhsT=b1_t[:1, :],
                                         rhs=ones_t[:1, :],
                                         start=False, stop=True)
                        nc.scalar.activation(
                            out=h1T_t[:, b * BLK:(b + 1) * BLK],
                            in_=o_ps[:], func=ident.Relu)

            def l1_pairs_only():
                # gathers + indicator + pair matmuls + acc drain, no tails
                for s in range(NSUP):
                    if s == 0:
                        gathers(s, xfull_d)
                    if s + 1 < NSUP:
                        gathers(s + 1, xfull_d)
                    buf = stage[s % 2]
                    for b in range(s * SBK, (s + 1) * SBK):
                        pl = pairs[b]
                        acc = ps_acc.tile([F_IN, BLK], dt.float32, tag="acc")
                        for j, (loc, pcol) in enumerate(pl):
                            ind = indp.tile([128, BLK], BF16, tag="ind")
                            nc.vector.tensor_scalar(
                                out=ind[:], in0=iota_t[:],
                                scalar1=seg_t[:, pcol:pcol + 1],
                                scalar2=rv_t[:, pcol:pcol + 1],
                                op0=mybir.AluOpType.is_equal,
                                op1=mybir.AluOpType.mult)
                            nc.tensor.matmul(
                                acc[:],
                                lhsT=buf[:, loc * F_IN:loc * F_IN + F_IN],
                                rhs=ind[:],
                                start=(j == 0), stop=(j == len(pl) - 1))
                        meanT = op.tile([F_IN, BLK], BF16, tag="meanT")
                        nc.scalar.activation(out=meanT[:], in_=acc[:],
                                             func=ident.Copy)

            def layer_restr(li, table, FW, wl_t, wr_t, bias_t, selfT,
                            out_sb):
                # all 7 pair-accumulations first, tails after
                for s in range(NSUP):
                    if s == 0:
                        gathers(s, table)
                    if s + 1 < NSUP:
                        gathers(s + 1, table)
                    buf = stage[s % 2]
                    accs = []
                    for b in range(s * SBK, (s + 1) * SBK):
                        pl = pairs[b]
                        acc = ps_acc.tile([FW, BLK], dt.float32, tag="acc")
                        accs.append(acc)
                        for j, (loc, pcol) in enumerate(pl):
                            ind = indp.tile([128, BLK], BF16, tag="ind")
                            nc.vector.tensor_scalar(
                                out=ind[:], in0=iota_t[:],
                                scalar1=seg_t[:, pcol:pcol + 1],
                                scalar2=rv_t[:, pcol:pcol + 1],
                                op0=mybir.AluOpType.is_equal,
                                op1=mybir.AluOpType.mult)
                            nc.tensor.matmul(
                                acc[:],
                                lhsT=buf[:, loc * F_IN:loc * F_IN + FW],
                                rhs=ind[:],
                                start=(j == 0), stop=(j == len(pl) - 1))
                    for bi, b in enumerate(range(s * SBK, (s + 1) * SBK)):
                        acc = accs[bi]
                        meanT = op.tile([FW, BLK], BF16, tag="meanT")
                        nc.scalar.activation(out=meanT[:], in_=acc[:],
                                             func=ident.Copy)
                        o_ps = ps_o.tile([F_OUT, BLK], dt.float32, tag="ops")
                        nc.tensor.matmul(o_ps[:], lhsT=wl_t[:], rhs=meanT[:],
                                         start=True, stop=False)
                        nc.tensor.matmul(o_ps[:], lhsT=wr_t[:],
                                         rhs=selfT[:, b * BLK:(b + 1) * BLK],
                                         start=False, stop=False)
                        nc.tensor.matmul(o_ps[:], lhsT=bias_t[:1, :],
                                         rhs=ones_t[:1, :],
                                         start=False, stop=True)
                        if li == 1:
                            nc.scalar.activation(
                                out=out_sb[:, b * BLK:(b + 1) * BLK],
                                in_=o_ps[:], func=ident.Relu)
                            tr = ps_t.tile([BLK, F_OUT], BF16, tag="tr")
                            nc.tensor.transpose(
                                out=tr[:],
                                in_=out_sb[:, b * BLK:(b + 1) * BLK],
                                identity=id_t[:])
                            h1n = op.tile([BLK, F_OUT], BF16, tag="h1n")
                            nc.scalar.activation(out=h1n[:], in_=tr[:],
                                                 func=ident.Copy)
                            nc.sync.dma_start(
                                h1pad_d[b * BLK:(b + 1) * BLK, :F_OUT],
                                h1n[:])
                        else:
                            ob = op.tile([F_OUT, BLK], dt.float32, tag="ob")
                            nc.scalar.activation(out=ob[:], in_=o_ps[:],
                                                 func=ident.Copy)
                            nc.sync.dma_start(
                                out_d[:, b * BLK:(b + 1) * BLK], ob[:])

            for _r in range(rep):
                if mode == "g1":
                    gathers_only()
                    continue
                if mode == "l1p":
                    l1_pairs_only()
                    continue
                if mode == "l1w":
                    # gathers + ALL pair matmuls, but const rhs (no DVE)
                    for s in range(NSUP):
                        if s == 0:
                            gathers(s, xfull_d)
                        if s + 1 < NSUP:
                            gathers(s + 1, xfull_d)
                        buf = stage[s % 2]
                        for b in range(s * SBK, (s + 1) * SBK):
                            pl = pairs[b]
                            acc = ps_acc.tile([F_IN, BLK], dt.float32,
                                              tag="acc")
                            for j, (loc, pcol) in enumerate(pl):
                                nc.tensor.matmul(
                                    acc[:],
                                    lhsT=buf[:, loc * F_IN:
                                             loc * F_IN + F_IN],
                                    rhs=iota_t[:],
                                    start=(j == 0), stop=(j == len(pl) - 1))
                            meanT = op.tile([F_IN, BLK], BF16, tag="meanT")
                            nc.scalar.activation(out=meanT[:], in_=acc[:],
                                                 func=ident.Copy)
                    continue
                if mode == "l1t":
                    # gathers + one consumer matmul per call (forces drain,
                    # minimal consumer instruction count)
                    for s in range(NSUP):
                        if s == 0:
                            gathers(s, xfull_d)
                        if s + 1 < NSUP:
                            gathers(s + 1, xfull_d)
                        buf = stage[s % 2]
                        acc = ps_acc.tile([F_IN, BLK], dt.float32, tag="acc")
                        scalls = [c for c in calls if c[0] == s]
                        for j, (ss, c, loc0, cols, ioff) in enumerate(scalls):
                            nc.tensor.matmul(
                                acc[:],
                                lhsT=buf[:, loc0 * F_IN:loc0 * F_IN + F_IN],
                                rhs=iota_t[:],
                                start=(j == 0), stop=(j == len(scalls) - 1))
                        meanT = op.tile([F_IN, BLK], BF16, tag="meanT")
                        nc.scalar.activation(out=meanT[:], in_=acc[:],
                                             func=ident.Copy)
                    continue
                if mode == "g1s":
                    for s in range(NSUP):
                        gathers(s, xfull_d, into=dummy)
                    continue
                if mode == "gser":
                    # identical gather stream into dummy bufs + identical
                    # compute on (never-written) real bufs: no data deps
                    for s in range(NSUP):
                        gathers(s, xfull_d, into=dummy)
                    compute_only()
                    continue
                if mode == "l1r":
                    layer_restr(1, xfull_d, F_IN, wl1_t, wr1_t, b1_t,
                                xt_t, h1T_t)
                    continue
                if mode == "c1":
                    compute_only()
                    continue
                if mode == "c1ns":
                    compute_nostore()
                    continue
                if mode == "agonly":
                    nc.gpsimd.collective_compute(
                        "AllGather", mybir.AluOpType.bypass,
                        replica_groups=[list(range(P))],
                        ins=[h1pad_d[:]], outs=[h1full_d[:]])
                    continue
                layer(1, xfull_d, F_IN, wl1_t, wr1_t, b1_t, xt_t, h1T_t)
                if mode == "l1":
                    nc.sync.dma_start(
                        out_d.bitcast(BF16)[:, :NL], h1T_t[:])
                    continue
                nc.gpsimd.collective_compute(
                    "AllGather", mybir.AluOpType.bypass,
                    replica_groups=[list(range(P))],
                    ins=[h1pad_d[:]], outs=[h1full_d[:]])
                if mode == "l1+ag":
                    nc.sync.dma_start(
                        out_d.bitcast(BF16)[:, :NL], h1T_t[:])
                    continue
                layer(2, h1full_d, F_OUT, wl2_t, wr2_t, b2_t, h1T_t, None)
            if mode in ("g1", "agonly", "l1p", "g1s", "l1t", "l1w"):
                nc.sync.dma_start(
                    out_d.bitcast(BF16)[:, :NL], xt_t[:F_OUT, :])
            elif mode in ("c1", "c1ns", "l1r", "gser"):
                nc.sync.dma_start(
                    out_d.bitcast(BF16)[:, :NL], h1T_t[:])

    nc.finalize()
    return nc


ICH = 40   # pairs per bulk-indicator build


def _build3(meta, rep=1, mode="full"):
    """v3: bulk indicator builds (tensor_tensor is_equal over broadcast APs)
    + node-major accumulation acc[dst,F] = ind01^T @ stage with 1/deg as a
    per-partition Act scale at PSUM drain. ~40 DVE instructions per layer
    instead of ~2200 (per-pair tensor_scalar interleaved with SWDGE gathers
    measured ~450ns each of sem/dispatch poison; bulk builds sidestep it).
    """
    calls = meta["calls"]
    pairs = meta["pairs"]
    MAXSUPC = meta["MAXSUPC"]
    NPAIR = meta["NPAIR"]
    IDX_COLS = meta["IDX_COLS"]

    nc = bacc.Bacc("TRN2", target_bir_lowering=False, debug=False,
                   num_devices=P, num_swdge_queues=4)
    dt = mybir.dt
    xfull_d = nc.dram_tensor("xfull", [NG, F_IN], BF16, kind="ExternalInput")
    xt_d = nc.dram_tensor("xt", [F_IN, NL], BF16, kind="ExternalInput")
    idx_d = nc.dram_tensor("idx", [128, IDX_COLS], dt.int16,
                           kind="ExternalInput")
    seg_d = nc.dram_tensor("seg", [128, NPAIR], BF16, kind="ExternalInput")
    rvn_d = nc.dram_tensor("rvn", [128, NB], dt.float32,
                           kind="ExternalInput")
    iota_d = nc.dram_tensor("iota", [128, BLK], BF16, kind="ExternalInput")
    wl1_d = nc.dram_tensor("W_l1", [F_IN, F_OUT], BF16, kind="ExternalInput")
    wr1_d = nc.dram_tensor("W_r1", [F_IN, F_OUT], BF16, kind="ExternalInput")
    b1_d = nc.dram_tensor("b1", [1, F_OUT], BF16, kind="ExternalInput")
    wl2_d = nc.dram_tensor("W_l2", [F_OUT, F_OUT], BF16,
                           kind="ExternalInput")
    wr2_d = nc.dram_tensor("W_r2", [F_OUT, F_OUT], BF16,
                           kind="ExternalInput")
    b2_d = nc.dram_tensor("b2", [1, F_OUT], BF16, kind="ExternalInput")
    out_d = nc.dram_tensor("out", [NL, F_OUT], dt.float32,
                           kind="ExternalOutput")

    h1pad_d = nc.dram_tensor("h1pad", [NL, F_IN], BF16)
    h1full_d = nc.dram_tensor("h1full", [NG, F_IN], BF16, addr_space="Shared")

    # super s covers pair range [srange[s], srange[s+1])
    srange = [pairs[s * SBK][0][1] for s in range(NSUP)] + [NPAIR]

    with tile.TileContext(nc) as tc:
        with (
            tc.tile_pool(name="const", bufs=1) as constp,
            tc.tile_pool(name="indall", bufs=3) as indall,
            tc.tile_pool(name="op", bufs=4) as op,
            tc.tile_pool(name="ps_acc", bufs=2, space="PSUM") as ps_acc,
            tc.tile_pool(name="ps_t", bufs=2, space="PSUM") as ps_t,
            tc.tile_pool(name="ps_o", bufs=2, space="PSUM") as ps_o,
        ):
            iota_t = constp.tile([128, BLK], BF16)
            nc.sync.dma_start(iota_t[:], iota_d[:])
            seg_t = constp.tile([128, NPAIR], BF16)
            nc.sync.dma_start(seg_t[:], seg_d[:])
            rvn_t = constp.tile([128, NB], dt.float32)
            nc.sync.dma_start(rvn_t[:], rvn_d[:])
            idx_t = constp.tile([128, IDX_COLS], dt.int16)
            nc.sync.dma_start(idx_t[:], idx_d[:])
            xt_t = constp.tile([F_IN, NL], BF16)
            nc.sync.dma_start(xt_t[:], xt_d[:])
            wl1_t = constp.tile([F_IN, F_OUT], BF16)
            nc.sync.dma_start(wl1_t[:], wl1_d[:])
            wr1_t = constp.tile([F_IN, F_OUT], BF16)
            nc.sync.dma_start(wr1_t[:], wr1_d[:])
            wl2_t = constp.tile([F_OUT, F_OUT], BF16)
            nc.sync.dma_start(wl2_t[:], wl2_d[:])
            wr2_t = constp.tile([F_OUT, F_OUT], BF16)
            nc.sync.dma_start(wr2_t[:], wr2_d[:])
            b1_t = constp.tile([1, F_OUT], BF16)
            nc.sync.dma_start(b1_t[:], b1_d[:])
            b2_t = constp.tile([1, F_OUT], BF16)
            nc.sync.dma_start(b2_t[:], b2_d[:])
            ones_t = constp.tile([1, BLK], BF16)
            nc.vector.memset(ones_t[:], 1.0)
            from concourse.masks import make_identity
            id128_t = constp.tile([128, 128], BF16)
            make_identity(nc, id128_t[:])
            h1T_t = constp.tile([F_OUT, NL], BF16)

            NSB = 3
            stage_a = constp.tile([128, MAXSUPC * F_IN], BF16)
            stage_b = constp.tile([128, MAXSUPC * F_IN], BF16)
            stage_c = constp.tile([128, MAXSUPC * F_IN], BF16)
            stage = [stage_a, stage_b, stage_c]
            for st_ in stage:
                nc.gpsimd.memset(st_[:], 0.0)

            qn = [0]
            ident = mybir.ActivationFunctionType

            def gathers(s, table):
                buf = stage[s % NSB]
                for (ss, c, loc0, cols, ioff) in calls:
                    if ss != s:
                        continue
                    nc.gpsimd.dma_gather(
                        out_ap=buf[:, loc0 * F_IN:(loc0 + cols) * F_IN]
                            .rearrange("p (c f) -> p c f", f=F_IN),
                        in_ap=table[c * CHUNK:min((c + 1) * CHUNK, NG), :],
                        idxs_ap=idx_t[:, ioff:ioff + cols * 8],
                        num_idxs=cols * 128, num_idxs_reg=cols * 128,
                        elem_size=F_IN, single_packet=True,
                        queue_num=qn[0] % 4)
                    qn[0] += 1

            def ind_builds(s):
                # bulk 0/1 indicators for all pairs of super s
                plo, phi = srange[s], srange[s + 1]
                cmap = {}
                for c0 in range(plo, phi, ICH):
                    np_ = min(ICH, phi - c0)
                    ich = indall.tile([128, ICH * 128], BF16, tag="ich")
                    nc.vector.tensor_tensor(
                        out=ich[:, :np_ * 128]
                            .rearrange("p (n j) -> p n j", j=128),
                        in0=seg_t[:, c0:c0 + np_].unsqueeze(2)
                            .broadcast_to([128, np_, 128]),
                        in1=iota_t[:].unsqueeze(1)
                            .broadcast_to([128, np_, BLK]),
                        op=mybir.AluOpType.is_equal)
                    for p in range(c0, c0 + np_):
                        cmap[p] = (ich, p - c0)
                return cmap

            def layer(li, table, FW, wl_t, wr_t, bias_t, selfT,
                      ag_hook=None):
                for s in range(NSUP):
                    if s == 0:
                        gathers(s, table)
                        if NSUP > 1:
                            gathers(1, table)
                    if s + 2 < NSUP:
                        gathers(s + 2, table)
                    cmap = ind_builds(s)
                    buf = stage[s % NSB]
                    for b in range(s * SBK, (s + 1) * SBK):
                        pl = pairs[b]
                        acc = ps_acc.tile([BLK, FW], dt.float32, tag="acc")
                        for j, (loc, pcol) in enumerate(pl):
                            it, off = cmap[pcol]
                            nc.tensor.matmul(
                                acc[:],
                                lhsT=it[:, off * 128:(off + 1) * 128],
                                rhs=buf[:, loc * F_IN:loc * F_IN + FW],
                                start=(j == 0), stop=(j == len(pl) - 1))
                        meanN = op.tile([BLK, FW], BF16, tag="meanN")
                        nc.scalar.activation(out=meanN[:], in_=acc[:],
                                             func=ident.Copy,
                                             scale=rvn_t[:, b:b + 1])
                        trm = ps_t.tile([FW, BLK], BF16, tag="trm")
                        nc.tensor.transpose(out=trm[:], in_=meanN[:],
                                            identity=id128_t[:])
                        meanT = op.tile([FW, BLK], BF16, tag="meanT")
                        nc.scalar.activation(out=meanT[:], in_=trm[:],
                                             func=ident.Copy)
                        o2 = ps_o.tile([BLK, F_OUT], dt.float32, tag="o2")
                        nc.tensor.matmul(o2[:], lhsT=meanT[:], rhs=wl_t[:],
                                         start=True, stop=False)
                        nc.tensor.matmul(o2[:],
                                         lhsT=selfT[:, b * BLK:(b + 1) * BLK],
                                         rhs=wr_t[:],
                                         start=False, stop=False)
                        nc.tensor.matmul(o2[:], lhsT=ones_t[:1, :],
                                         rhs=bias_t[:1, :],
                                         start=False, stop=True)
                        if li == 1:
                            h1n = op.tile([BLK, F_OUT], BF16, tag="h1n")
                            nc.scalar.activation(out=h1n[:], in_=o2[:],
                                                 func=ident.Relu)
                            nc.sync.dma_start(
                                h1pad_d[b * BLK:(b + 1) * BLK, :F_OUT],
                                h1n[:])
                            trh = ps_t.tile([F_OUT, BLK], BF16, tag="trh")
                            nc.tensor.transpose(out=trh[:], in_=h1n[:],
                                                identity=id128_t[:])
                            nc.scalar.activation(
                                out=h1T_t[:, b * BLK:(b + 1) * BLK],
                                in_=trh[:], func=ident.Copy)
                        else:
                            ob = op.tile([BLK, F_OUT], dt.float32, tag="ob")
                            nc.scalar.activation(out=ob[:], in_=o2[:],
                                                 func=ident.Copy)
                            nc.sync.dma_start(
                                out_d[b * BLK:(b + 1) * BLK, :], ob[:])
                    if ag_hook is not None:
                        ag_hook(s)

            def ag_group(g):
                a = g * GRP_ROWS
                nc.gpsimd.collective_compute(
                    "AllGather", mybir.AluOpType.bypass,
                    replica_groups=[list(range(P))],
                    ins=[h1pad_d[a:a + GSZ[g], :]],
                    outs=[h1full_d[GBASE[g]:GBASE[g] + P * GSZ[g], :]])

            for _r in range(rep):
                bounds = list(range(AGS, NSUP, AGS)) + [NSUP]

                def hook(s):
                    if s + 1 in bounds:
                        ag_group(bounds.index(s + 1))

                layer(1, xfull_d, F_IN, wl1_t, wr1_t, b1_t, xt_t,
                      ag_hook=(None if mode == "l1" else hook))
                if mode == "l1" or mode == "l1ag":
                    continue
                layer(2, h1full_d, F_OUT, wl2_t, wr2_t, b2_t, h1T_t)


    nc.finalize()
    return nc


def _make_inputs3(x, W_l1, W_r1, b1, W_l2, W_r2, b2, meta):
    x = np.asarray(x, dtype=np.float32)
    x_full = np.zeros((NG, F_IN), dtype=np.float32)
    slots = np.arange(NREAL)
    for k in range(P):
        x_full[_gmap(k, slots)] = x[k * NREAL:(k + 1) * NREAL]
    x_full_bf = x_full.astype(NPBF16)
    xpad = np.zeros((P, NL, F_IN), dtype=np.float32)
    for k in range(P):
        xpad[k, :NREAL] = x[k * NREAL:(k + 1) * NREAL]
    xpad_bf = xpad.astype(NPBF16)
    iota = np.broadcast_to(np.arange(BLK, dtype=np.float32),
                           (128, BLK)).astype(NPBF16).copy()
    in_maps = []
    for k in range(P):
        in_maps.append({
            "xfull": x_full_bf,
            "xt": np.ascontiguousarray(xpad_bf[k].T),
            "idx": meta["idx"][k],
            "seg": meta["segb"][k],
            "rvn": meta["rvn"][k],
            "iota": iota,
            "W_l1": np.asarray(W_l1, np.float32).astype(NPBF16),
            "W_r1": np.asarray(W_r1, np.float32).astype(NPBF16),
            "b1": np.asarray(b1, np.float32).reshape(1, F_OUT).astype(NPBF16),
            "W_l2": np.asarray(W_l2, np.float32).astype(NPBF16),
            "W_r2": np.asarray(W_r2, np.float32).astype(NPBF16),
            "b2": np.asarray(b2, np.float32).reshape(1, F_OUT).astype(NPBF16),
        })
    return in_maps


def _make_inputs(x, W_l1, W_r1, b1, W_l2, W_r2, b2, meta):
    x = np.asarray(x, dtype=np.float32)
    x_full = np.zeros((NG, F_IN), dtype=np.float32)
    for k in range(P):
        x_full[k * NL:k * NL + NREAL] = x[k * NREAL:(k + 1) * NREAL]
    x_full_bf = x_full.astype(NPBF16)
    iota = np.broadcast_to(np.arange(BLK, dtype=np.float32),
                           (128, BLK)).astype(NPBF16).copy()
    in_maps = []
    for k in range(P):
        in_maps.append({
            "xfull": x_full_bf,
            "xt": np.ascontiguousarray(x_full_bf[k * NL:(k + 1) * NL].T),
            "idx": meta["idx"][k],
            "seg": meta["seg"][k],
            "rv": meta["rv"][k],
            "iota": iota,
            "W_l1": np.asarray(W_l1, np.float32).astype(NPBF16),
            "W_r1": np.asarray(W_r1, np.float32).astype(NPBF16),
            "b1": np.asarray(b1, np.float32).reshape(1, F_OUT).astype(NPBF16),
            "W_l2": np.asarray(W_l2, np.float32).astype(NPBF16),
            "W_r2": np.asarray(W_r2, np.float32).astype(NPBF16),
            "b2": np.asarray(b2, np.float32).reshape(1, F_OUT).astype(NPBF16),
        })
    return in_maps


def kernel(x, edge_index, W_l1, W_r1, b1, W_l2, W_r2, b2):
    meta = _preprocess(np.asarray(edge_index))
    in_maps = _make_inputs3(x, W_l1, W_r1, b1, W_l2, W_r2, b2, meta)
    nc = _build3(meta)
    res = run_bass_kernel_spmd(nc, in_maps, core_ids=list(range(P)))
    out = np.concatenate(
        [res.results[k]["out"][:NREAL] for k in range(P)], axis=0)
    return out.astype(np.float32)


if __name__ == "__main__":
    rng = np.random.default_rng(0)
    x = rng.normal(size=(N_NODES, F_IN)).astype(np.float32)
    ei = rng.integers(0, N_NODES, size=(2, N_EDGES)).astype(np.int64)
    wl1 = rng.normal(size=(F_IN, F_OUT)).astype(np.float32) / np.sqrt(F_IN)
    wr1 = rng.normal(size=(F_IN, F_OUT)).astype(np.float32) / np.sqrt(F_IN)
    wl2 = rng.normal(size=(F_OUT, F_OUT)).astype(np.float32) / np.sqrt(F_OUT)
    wr2 = rng.normal(size=(F_OUT, F_OUT)).astype(np.float32) / np.sqrt(F_OUT)
    b1 = np.zeros(F_OUT, np.float32)
    b2 = np.zeros(F_OUT, np.float32)
    out = kernel(x, ei, wl1, wr1, b1, wl2, wr2, b2)
    print("out", out.shape, out.dtype, float(np.abs(out).mean()))

